# revision 7
# baseline (speedup 1.0000x reference)
"""LoTD forest encoding (NGP-style multi-level hash grid, 8-tree forest) on TRN2.

Data-parallel over points across 8 NeuronCores. Per core, a hardware loop
(Tile For_i) processes batches of 1024 points:
  - DVE computes the spatial-hash table rows for all 8 trilinear corners of
    each point at each of the 16 levels (fp32-exact arithmetic for the
    mod-2^17 multiplies, int32 xor/and for the hash combine),
  - the per-corner feature pairs are fetched from a replicated bf16 copy of
    the full [8 trees x 16 levels x 131072 x 2] table in device DRAM via
    SWDGE indirect DMA (128 offsets per instruction, one per partition),
  - DVE applies the trilinear corner weights and accumulates the [N, 32]
    output, streamed back per batch.

The hash h = (x ^ y*P1 ^ z*P2) & (2^17-1) is computed without 32-bit integer
multiplies: only P mod 2^17 matters, and y*(P mod 2^17) is split into
(y%32)*(P mod 2^17) + (y//32)*((32*P) mod 2^17), both exact in fp32.

float->int casts on the vector engine round to nearest, so floor(pos) is
computed as cast(pos - 0.5), with the fractional weight w = pos - float(ip);
an off-by-one at exact-integer pos yields w=1.0 and the identical
interpolation result.
"""

import numpy as np

L = 16
F = 2
T = 1 << 17
N_TREES = 8
N = 2_000_000
RES = np.array([16, 22, 30, 41, 55, 75, 102, 139, 188, 256, 348, 472,
                642, 872, 1184, 1608], dtype=np.int64)
P1 = 2654435761
P2 = 805459861
MASK = T - 1
K1 = P1 & MASK            # y multiplier mod 2^17
K1H = (32 * P1) & MASK
K2 = P2 & MASK            # z multiplier mod 2^17
K2H = (32 * P2) & MASK

NCORES = 8
B = 2048                  # points per batch
NCOL = B // 128           # 8 point-columns per partition
NBATCH = 123              # batches per core
NPC = NBATCH * B          # 250880 points per core (padded)

_CACHE = {}


def _build_nc():
    import concourse.bass as bass
    import concourse.bacc as bacc
    import concourse.mybir as mybir
    import concourse.tile as tile
    from concourse.bass import IndirectOffsetOnAxis

    fp32 = mybir.dt.float32
    int32 = mybir.dt.int32
    bf16 = mybir.dt.bfloat16
    AO = mybir.AluOpType

    nc = bacc.Bacc("TRN2")
    # xb[b*128+p, d*NCOL+j] = x of point (b, j*128+p), dim d
    xb = nc.dram_tensor("xb", [NBATCH * 128, 3 * NCOL], fp32, kind="ExternalInput")
    # tb[b*128+p, j] = tree id (int32)
    tb = nc.dram_tensor("tb", [NBATCH * 128, NCOL], int32, kind="ExternalInput")
    # ftab[(tree*16+l)*T + h] = feature pair (bf16)
    ftab = nc.dram_tensor("ftab", [N_TREES * L * T, F], fp32, kind="ExternalInput")
    out = nc.dram_tensor("out", [NPC, 2 * L], fp32, kind="ExternalOutput")

    with tile.TileContext(nc) as tc:
        with tc.tile_pool(name="sbuf", bufs=1) as pool:
            x_t = pool.tile([128, 3 * NCOL], fp32, tag="x")
            t_i = pool.tile([128, NCOL], int32, tag="ti")
            tbase = pool.tile([128, NCOL], int32, tag="tbase")
            pos = pool.tile([128, 3 * NCOL], fp32, tag="pos")
            ipi = pool.tile([128, 3 * NCOL], int32, tag="ipi")
            ipf = pool.tile([128, 3 * NCOL], fp32, tag="ipf")
            w3 = pool.tile([128, 3 * NCOL], fp32, tag="w3")
            w3m = pool.tile([128, 3 * NCOL], fp32, tag="w3m")     # 1 - w
            hh = pool.tile([128, NCOL], fp32, tag="hh")           # scratch f
            hl = pool.tile([128, NCOL], fp32, tag="hl")
            tyf = pool.tile([128, 2 * NCOL], fp32, tag="tyf")     # y,z terms f
            tyi = pool.tile([128, 2 * NCOL], int32, tag="tyi")    # y0,z0 int
            tyi1 = pool.tile([128, 2 * NCOL], int32, tag="tyi1")  # y1,z1 int
            ix1 = pool.tile([128, NCOL], int32, tag="ix1")
            xy = pool.tile([128, 4 * NCOL], int32, tag="xy")      # x^y for 4 combos
            hc = pool.tile([128, NCOL], int32, tag="hc")          # corner hash scratch
            idx_a = pool.tile([128, 8 * NCOL], int32, tag="idxA")
            idx_b = pool.tile([128, 8 * NCOL], int32, tag="idxB")
            g_a = pool.tile([128, 16 * NCOL], fp32, tag="gA")
            g_b = pool.tile([128, 16 * NCOL], fp32, tag="gB")
            idx_ab = [idx_a, idx_b]
            g_ab = [g_a, g_b]
            wyz = pool.tile([128, 4 * NCOL], fp32, tag="wyz")
            w8 = pool.tile([128, 8 * NCOL], fp32, tag="w8")
            w16 = pool.tile([128, 16 * NCOL], fp32, tag="w16")
            tsum = pool.tile([128, 2 * NCOL], fp32, tag="tsum")
            acc = pool.tile([128, NCOL, 2 * L], fp32, tag="acc")

            with tc.For_i(0, NBATCH) as bi:
                nc.sync.dma_start(out=x_t[:], in_=xb[bass.ts(bi, 128), :])
                nc.sync.dma_start(out=t_i[:], in_=tb[bass.ts(bi, 128), :])
                # tbase = tree * (16*T)
                nc.vector.tensor_scalar(out=tbase[:], in0=t_i[:], scalar1=L * T,
                                        scalar2=0, op0=AO.mult, op1=AO.add)
                for l in range(L):
                    idx_t = idx_ab[l % 2]
                    g = g_ab[l % 2]
                    R = int(RES[l])
                    s = (R - 1) * 0.5
                    # pos = x*s + s ; ip = round(pos-0.5) ; w = pos - ip
                    nc.vector.tensor_scalar(out=pos[:], in0=x_t[:], scalar1=s,
                                            scalar2=s, op0=AO.mult, op1=AO.add)
                    nc.vector.tensor_scalar(out=ipf[:], in0=pos[:], scalar1=1.0,
                                            scalar2=-0.5, op0=AO.mult, op1=AO.add)
                    nc.vector.tensor_copy(out=ipi[:], in_=ipf[:])
                    nc.vector.tensor_copy(out=ipf[:], in_=ipi[:])
                    nc.vector.tensor_tensor(out=w3[:], in0=pos[:], in1=ipf[:],
                                            op=AO.subtract)
                    nc.vector.tensor_scalar(out=w3m[:], in0=w3[:], scalar1=-1.0,
                                            scalar2=1.0, op0=AO.mult, op1=AO.add)
                    # y/z hash terms: t = (i%32)*K + (i//32)*KH  (exact fp32)
                    for d, (KA, KB) in ((1, (K1, K1H)), (2, (K2, K2H))):
                        src = ipf[:, d * NCOL:(d + 1) * NCOL]
                        nc.vector.tensor_scalar(out=hh[:], in0=src, scalar1=0.03125,
                                                scalar2=-0.5, op0=AO.mult, op1=AO.add)
                        nc.vector.tensor_copy(out=hc[:], in_=hh[:])      # int floor
                        nc.vector.tensor_copy(out=hh[:], in_=hc[:])      # back to f
                        nc.vector.tensor_scalar(out=hl[:], in0=hh[:], scalar1=-32.0,
                                                scalar2=0.0, op0=AO.mult, op1=AO.add)
                        nc.vector.tensor_tensor(out=hl[:], in0=src, in1=hl[:],
                                                op=AO.add)               # i%32
                        nc.vector.tensor_scalar(out=hl[:], in0=hl[:], scalar1=float(KA),
                                                scalar2=0.0, op0=AO.mult, op1=AO.add)
                        nc.vector.tensor_scalar(out=hh[:], in0=hh[:], scalar1=float(KB),
                                                scalar2=0.0, op0=AO.mult, op1=AO.add)
                        dst = tyf[:, (d - 1) * NCOL:d * NCOL]
                        nc.vector.tensor_tensor(out=dst, in0=hl[:], in1=hh[:], op=AO.add)
                    nc.vector.tensor_copy(out=tyi[:], in_=tyf[:])
                    # +K for the +1 corners
                    nc.vector.tensor_scalar(out=tyi1[:, :NCOL], in0=tyi[:, :NCOL],
                                            scalar1=K1, scalar2=0, op0=AO.add, op1=AO.add)
                    nc.vector.tensor_scalar(out=tyi1[:, NCOL:], in0=tyi[:, NCOL:],
                                            scalar1=K2, scalar2=0, op0=AO.add, op1=AO.add)
                    nc.vector.tensor_scalar(out=ix1[:], in0=ipi[:, :NCOL], scalar1=1,
                                            scalar2=0, op0=AO.add, op1=AO.add)
                    # xy[dx*2+dy] = ix_dx ^ ty_dy
                    for dx, xsrc in ((0, ipi[:, :NCOL]), (1, ix1[:])):
                        for dy, ysrc in ((0, tyi[:, :NCOL]), (1, tyi1[:, :NCOL])):
                            nc.vector.tensor_tensor(
                                out=xy[:, (dx * 2 + dy) * NCOL:(dx * 2 + dy + 1) * NCOL],
                                in0=xsrc, in1=ysrc, op=AO.bitwise_xor)
                    # corners c = dx*4 + dy*2 + dz (matches OFFS ordering)
                    for dx in range(2):
                        for dy in range(2):
                            for dz in range(2):
                                c = dx * 4 + dy * 2 + dz
                                zsrc = tyi[:, NCOL:] if dz == 0 else tyi1[:, NCOL:]
                                nc.vector.tensor_tensor(
                                    out=hc[:],
                                    in0=xy[:, (dx * 2 + dy) * NCOL:(dx * 2 + dy + 1) * NCOL],
                                    in1=zsrc, op=AO.bitwise_xor)
                                nc.vector.tensor_scalar(out=hc[:], in0=hc[:],
                                                        scalar1=MASK, scalar2=l * T,
                                                        op0=AO.bitwise_and, op1=AO.bitwise_or)
                                nc.vector.tensor_tensor(
                                    out=idx_t[:, c * NCOL:(c + 1) * NCOL],
                                    in0=hc[:], in1=tbase[:], op=AO.add)
                    # gathers: one 128-offset indirect DMA per (corner, column)
                    for q in range(8 * NCOL):
                        nc.gpsimd.indirect_dma_start(
                            out=g[:, 2 * q:2 * q + 2],
                            out_offset=None,
                            in_=ftab[:],
                            in_offset=IndirectOffsetOnAxis(ap=idx_t[:, q:q + 1], axis=0),
                        )
                    # weights: w8[c] = wx_dx * wy_dy * wz_dz
                    for dy in range(2):
                        ws = w3m if dy == 0 else w3
                        for dz in range(2):
                            zs = w3m if dz == 0 else w3
                            nc.vector.tensor_tensor(
                                out=wyz[:, (dy * 2 + dz) * NCOL:(dy * 2 + dz + 1) * NCOL],
                                in0=ws[:, NCOL:2 * NCOL], in1=zs[:, 2 * NCOL:3 * NCOL],
                                op=AO.mult)
                    for dx in range(2):
                        xs = w3m if dx == 0 else w3
                        for k in range(4):
                            c = dx * 4 + k
                            nc.vector.tensor_tensor(
                                out=w8[:, c * NCOL:(c + 1) * NCOL],
                                in0=xs[:, :NCOL], in1=wyz[:, k * NCOL:(k + 1) * NCOL],
                                op=AO.mult)
                    # duplicate each weight across the 2 features
                    w16v = w16[:].rearrange("p (q two) -> p q two", two=2)
                    w8v = w8[:].rearrange("p (q one) -> p q one", one=1)
                    nc.vector.tensor_copy(out=w16v[:, :, 0:1], in_=w8v)
                    nc.vector.tensor_copy(out=w16v[:, :, 1:2], in_=w8v)
                    # weighted sum over corners
                    gf = g
                    nc.vector.tensor_tensor(out=gf[:], in0=gf[:], in1=w16[:], op=AO.mult)
                    nc.vector.tensor_tensor(out=tsum[:], in0=gf[:, :2 * NCOL],
                                            in1=gf[:, 2 * NCOL:4 * NCOL], op=AO.add)
                    for c in range(2, 8):
                        nc.vector.tensor_tensor(
                            out=tsum[:], in0=tsum[:],
                            in1=gf[:, c * 2 * NCOL:(c + 1) * 2 * NCOL], op=AO.add)
                    # tsum[p, j*2+f] -> acc[p, j, 2l+f]
                    nc.vector.tensor_copy(
                        out=acc[:, :, 2 * l:2 * l + 2],
                        in_=tsum[:].rearrange("p (j f) -> p j f", f=2))
                # out rows j*128+p <- acc[p, j, :]
                ov = out[bass.ts(bi, B), :].rearrange("(j p) f -> p j f", p=128)
                nc.sync.dma_start(out=ov, in_=acc[:])
    nc.compile()
    return nc


def _prep(block_x, params, block_inds):
    x = np.asarray(block_x, dtype=np.float32)
    inds = np.asarray(block_inds).astype(np.int32)
    n = x.shape[0]
    ntot = NCORES * NPC
    xp = np.zeros((ntot, 3), dtype=np.float32)
    xp[:n] = x
    ip = np.zeros(ntot, dtype=np.int32)
    ip[:n] = inds
    # [c, b, j, p, d] -> [c, b*128+p, d*NCOL+j]
    xr = xp.reshape(NCORES, NBATCH, NCOL, 128, 3).transpose(0, 1, 3, 4, 2)
    xr = np.ascontiguousarray(xr).reshape(NCORES, NBATCH * 128, 3 * NCOL)
    tr = ip.reshape(NCORES, NBATCH, NCOL, 128).transpose(0, 1, 3, 2)
    tr = np.ascontiguousarray(tr).reshape(NCORES, NBATCH * 128, NCOL)
    ftab = np.ascontiguousarray(np.asarray(params, dtype=np.float32).reshape(N_TREES * L * T, F))
    return xr, tr, ftab, n


def kernel(block_x, params, block_inds):
    from concourse.bass_utils import run_bass_kernel_spmd

    xr, tr, ftab, n = _prep(block_x, params, block_inds)
    if "nc" not in _CACHE:
        _CACHE["nc"] = _build_nc()
    nc = _CACHE["nc"]
    in_maps = [{"xb": xr[c], "tb": tr[c], "ftab": ftab} for c in range(NCORES)]
    res = run_bass_kernel_spmd(nc, in_maps, core_ids=list(range(NCORES)))
    outs = [res.results[c]["out"] for c in range(NCORES)]
    full = np.concatenate(outs, axis=0)[:n]
    return np.ascontiguousarray(full)


# revision 8
# speedup vs baseline: 1.0519x; 1.0519x over previous
"""LoTD forest encoding (NGP-style multi-level hash grid, 8-tree forest) on TRN2.

Data-parallel over points across 8 NeuronCores. Per core, a hardware loop
(Tile For_i) processes batches of 1024 points:
  - DVE computes the spatial-hash table rows for all 8 trilinear corners of
    each point at each of the 16 levels (fp32-exact arithmetic for the
    mod-2^17 multiplies, int32 xor/and for the hash combine),
  - the per-corner feature pairs are fetched from a replicated bf16 copy of
    the full [8 trees x 16 levels x 131072 x 2] table in device DRAM via
    SWDGE indirect DMA (128 offsets per instruction, one per partition),
  - DVE applies the trilinear corner weights and accumulates the [N, 32]
    output, streamed back per batch.

The hash h = (x ^ y*P1 ^ z*P2) & (2^17-1) is computed without 32-bit integer
multiplies: only P mod 2^17 matters, and y*(P mod 2^17) is split into
(y%32)*(P mod 2^17) + (y//32)*((32*P) mod 2^17), both exact in fp32.

float->int casts on the vector engine round to nearest, so floor(pos) is
computed as cast(pos - 0.5), with the fractional weight w = pos - float(ip);
an off-by-one at exact-integer pos yields w=1.0 and the identical
interpolation result.
"""

import numpy as np

L = 16
F = 2
T = 1 << 17
N_TREES = 8
N = 2_000_000
RES = np.array([16, 22, 30, 41, 55, 75, 102, 139, 188, 256, 348, 472,
                642, 872, 1184, 1608], dtype=np.int64)
P1 = 2654435761
P2 = 805459861
MASK = T - 1
K1 = P1 & MASK            # y multiplier mod 2^17
K1H = (32 * P1) & MASK
K2 = P2 & MASK            # z multiplier mod 2^17
K2H = (32 * P2) & MASK

NCORES = 8
B = 1024                  # points per batch
NCOL = B // 128           # 8 point-columns per partition
NBATCH = 245              # batches per core
NPC = NBATCH * B          # 250880 points per core (padded)

_CACHE = {}


def _build_nc():
    import concourse.bass as bass
    import concourse.bacc as bacc
    import concourse.mybir as mybir
    import concourse.tile as tile
    from concourse.bass import IndirectOffsetOnAxis

    fp32 = mybir.dt.float32
    int32 = mybir.dt.int32
    bf16 = mybir.dt.bfloat16
    AO = mybir.AluOpType

    nc = bacc.Bacc("TRN2")
    # xb[b*128+p, d*NCOL+j] = x of point (b, j*128+p), dim d
    xb = nc.dram_tensor("xb", [NBATCH * 128, 3 * NCOL], fp32, kind="ExternalInput")
    # tb[b*128+p, j] = tree id (int32)
    tb = nc.dram_tensor("tb", [NBATCH * 128, NCOL], int32, kind="ExternalInput")
    # ftab[(tree*16+l)*T + h] = feature pair (bf16)
    ftab = nc.dram_tensor("ftab", [N_TREES * L * T, F], fp32, kind="ExternalInput")
    out = nc.dram_tensor("out", [NPC, 2 * L], fp32, kind="ExternalOutput")

    with tile.TileContext(nc) as tc:
        with tc.tile_pool(name="sbuf", bufs=1) as pool:
            x_t = pool.tile([128, 3 * NCOL], fp32, tag="x")
            t_i = pool.tile([128, NCOL], int32, tag="ti")
            tbase = pool.tile([128, NCOL], int32, tag="tbase")
            pos = pool.tile([128, 3 * NCOL], fp32, tag="pos")
            ipi = pool.tile([128, 3 * NCOL], int32, tag="ipi")
            ipf = pool.tile([128, 3 * NCOL], fp32, tag="ipf")
            w3 = pool.tile([128, 3 * NCOL], fp32, tag="w3")
            w3m = pool.tile([128, 3 * NCOL], fp32, tag="w3m")     # 1 - w
            hh = pool.tile([128, NCOL], fp32, tag="hh")           # scratch f
            hl = pool.tile([128, NCOL], fp32, tag="hl")
            tyf = pool.tile([128, 2 * NCOL], fp32, tag="tyf")     # y,z terms f
            tyi = pool.tile([128, 2 * NCOL], int32, tag="tyi")    # y0,z0 int
            tyi1 = pool.tile([128, 2 * NCOL], int32, tag="tyi1")  # y1,z1 int
            ix1 = pool.tile([128, NCOL], int32, tag="ix1")
            xy = pool.tile([128, 4 * NCOL], int32, tag="xy")      # x^y for 4 combos
            hc = pool.tile([128, NCOL], int32, tag="hc")          # corner hash scratch
            idx_t = pool.tile([128, 8 * NCOL], int32, tag="idx")
            g = pool.tile([128, 16 * NCOL], fp32, tag="g")
            wyz = pool.tile([128, 4 * NCOL], fp32, tag="wyz")
            w8 = pool.tile([128, 8 * NCOL], fp32, tag="w8")
            w16 = pool.tile([128, 16 * NCOL], fp32, tag="w16")
            tsum = pool.tile([128, 2 * NCOL], fp32, tag="tsum")
            acc = pool.tile([128, NCOL, 2 * L], fp32, tag="acc")

            with tc.For_i(0, NBATCH) as bi:
                nc.sync.dma_start(out=x_t[:], in_=xb[bass.ts(bi, 128), :])
                nc.sync.dma_start(out=t_i[:], in_=tb[bass.ts(bi, 128), :])
                # tbase = tree * (16*T)
                nc.vector.tensor_scalar(out=tbase[:], in0=t_i[:], scalar1=L * T,
                                        scalar2=0, op0=AO.mult, op1=AO.add)
                for l in range(L):
                    R = int(RES[l])
                    s = (R - 1) * 0.5
                    # pos = x*s + s ; ip = round(pos-0.5) ; w = pos - ip
                    nc.vector.tensor_scalar(out=pos[:], in0=x_t[:], scalar1=s,
                                            scalar2=s, op0=AO.mult, op1=AO.add)
                    nc.vector.tensor_scalar(out=ipf[:], in0=pos[:], scalar1=1.0,
                                            scalar2=-0.5, op0=AO.mult, op1=AO.add)
                    nc.vector.tensor_copy(out=ipi[:], in_=ipf[:])
                    nc.vector.tensor_copy(out=ipf[:], in_=ipi[:])
                    nc.vector.tensor_tensor(out=w3[:], in0=pos[:], in1=ipf[:],
                                            op=AO.subtract)
                    nc.vector.tensor_scalar(out=w3m[:], in0=w3[:], scalar1=-1.0,
                                            scalar2=1.0, op0=AO.mult, op1=AO.add)
                    # y/z hash terms: t = (i%32)*K + (i//32)*KH  (exact fp32)
                    for d, (KA, KB) in ((1, (K1, K1H)), (2, (K2, K2H))):
                        src = ipf[:, d * NCOL:(d + 1) * NCOL]
                        nc.vector.tensor_scalar(out=hh[:], in0=src, scalar1=0.03125,
                                                scalar2=-0.5, op0=AO.mult, op1=AO.add)
                        nc.vector.tensor_copy(out=hc[:], in_=hh[:])      # int floor
                        nc.vector.tensor_copy(out=hh[:], in_=hc[:])      # back to f
                        nc.vector.tensor_scalar(out=hl[:], in0=hh[:], scalar1=-32.0,
                                                scalar2=0.0, op0=AO.mult, op1=AO.add)
                        nc.vector.tensor_tensor(out=hl[:], in0=src, in1=hl[:],
                                                op=AO.add)               # i%32
                        nc.vector.tensor_scalar(out=hl[:], in0=hl[:], scalar1=float(KA),
                                                scalar2=0.0, op0=AO.mult, op1=AO.add)
                        nc.vector.tensor_scalar(out=hh[:], in0=hh[:], scalar1=float(KB),
                                                scalar2=0.0, op0=AO.mult, op1=AO.add)
                        dst = tyf[:, (d - 1) * NCOL:d * NCOL]
                        nc.vector.tensor_tensor(out=dst, in0=hl[:], in1=hh[:], op=AO.add)
                    nc.vector.tensor_copy(out=tyi[:], in_=tyf[:])
                    # +K for the +1 corners
                    nc.vector.tensor_scalar(out=tyi1[:, :NCOL], in0=tyi[:, :NCOL],
                                            scalar1=K1, scalar2=0, op0=AO.add, op1=AO.add)
                    nc.vector.tensor_scalar(out=tyi1[:, NCOL:], in0=tyi[:, NCOL:],
                                            scalar1=K2, scalar2=0, op0=AO.add, op1=AO.add)
                    nc.vector.tensor_scalar(out=ix1[:], in0=ipi[:, :NCOL], scalar1=1,
                                            scalar2=0, op0=AO.add, op1=AO.add)
                    # xy[dx*2+dy] = ix_dx ^ ty_dy
                    for dx, xsrc in ((0, ipi[:, :NCOL]), (1, ix1[:])):
                        for dy, ysrc in ((0, tyi[:, :NCOL]), (1, tyi1[:, :NCOL])):
                            nc.vector.tensor_tensor(
                                out=xy[:, (dx * 2 + dy) * NCOL:(dx * 2 + dy + 1) * NCOL],
                                in0=xsrc, in1=ysrc, op=AO.bitwise_xor)
                    # corners c = dx*4 + dy*2 + dz (matches OFFS ordering)
                    for dx in range(2):
                        for dy in range(2):
                            for dz in range(2):
                                c = dx * 4 + dy * 2 + dz
                                zsrc = tyi[:, NCOL:] if dz == 0 else tyi1[:, NCOL:]
                                nc.vector.tensor_tensor(
                                    out=hc[:],
                                    in0=xy[:, (dx * 2 + dy) * NCOL:(dx * 2 + dy + 1) * NCOL],
                                    in1=zsrc, op=AO.bitwise_xor)
                                nc.vector.tensor_scalar(out=hc[:], in0=hc[:],
                                                        scalar1=MASK, scalar2=l * T,
                                                        op0=AO.bitwise_and, op1=AO.bitwise_or)
                                nc.vector.tensor_tensor(
                                    out=idx_t[:, c * NCOL:(c + 1) * NCOL],
                                    in0=hc[:], in1=tbase[:], op=AO.add)
                    # gathers: one 128-offset indirect DMA per (corner, column)
                    for q in range(8 * NCOL):
                        nc.gpsimd.indirect_dma_start(
                            out=g[:, 2 * q:2 * q + 2],
                            out_offset=None,
                            in_=ftab[:],
                            in_offset=IndirectOffsetOnAxis(ap=idx_t[:, q:q + 1], axis=0),
                        )
                    # weights: w8[c] = wx_dx * wy_dy * wz_dz
                    for dy in range(2):
                        ws = w3m if dy == 0 else w3
                        for dz in range(2):
                            zs = w3m if dz == 0 else w3
                            nc.vector.tensor_tensor(
                                out=wyz[:, (dy * 2 + dz) * NCOL:(dy * 2 + dz + 1) * NCOL],
                                in0=ws[:, NCOL:2 * NCOL], in1=zs[:, 2 * NCOL:3 * NCOL],
                                op=AO.mult)
                    for dx in range(2):
                        xs = w3m if dx == 0 else w3
                        for k in range(4):
                            c = dx * 4 + k
                            nc.vector.tensor_tensor(
                                out=w8[:, c * NCOL:(c + 1) * NCOL],
                                in0=xs[:, :NCOL], in1=wyz[:, k * NCOL:(k + 1) * NCOL],
                                op=AO.mult)
                    # duplicate each weight across the 2 features
                    w16v = w16[:].rearrange("p (q two) -> p q two", two=2)
                    w8v = w8[:].rearrange("p (q one) -> p q one", one=1)
                    nc.vector.tensor_copy(out=w16v[:, :, 0:1], in_=w8v)
                    nc.vector.tensor_copy(out=w16v[:, :, 1:2], in_=w8v)
                    # weighted sum over corners
                    gf = g
                    nc.vector.tensor_tensor(out=gf[:], in0=gf[:], in1=w16[:], op=AO.mult)
                    nc.vector.tensor_tensor(out=tsum[:], in0=gf[:, :2 * NCOL],
                                            in1=gf[:, 2 * NCOL:4 * NCOL], op=AO.add)
                    for c in range(2, 8):
                        nc.vector.tensor_tensor(
                            out=tsum[:], in0=tsum[:],
                            in1=gf[:, c * 2 * NCOL:(c + 1) * 2 * NCOL], op=AO.add)
                    # tsum[p, j*2+f] -> acc[p, j, 2l+f]
                    nc.vector.tensor_copy(
                        out=acc[:, :, 2 * l:2 * l + 2],
                        in_=tsum[:].rearrange("p (j f) -> p j f", f=2))
                # out rows j*128+p <- acc[p, j, :]
                ov = out[bass.ts(bi, B), :].rearrange("(j p) f -> p j f", p=128)
                nc.sync.dma_start(out=ov, in_=acc[:])
    nc.compile()
    return nc


def _prep(block_x, params, block_inds):
    x = np.asarray(block_x, dtype=np.float32)
    inds = np.asarray(block_inds).astype(np.int32)
    n = x.shape[0]
    ntot = NCORES * NPC
    xp = np.zeros((ntot, 3), dtype=np.float32)
    xp[:n] = x
    ip = np.zeros(ntot, dtype=np.int32)
    ip[:n] = inds
    # [c, b, j, p, d] -> [c, b*128+p, d*NCOL+j]
    xr = xp.reshape(NCORES, NBATCH, NCOL, 128, 3).transpose(0, 1, 3, 4, 2)
    xr = np.ascontiguousarray(xr).reshape(NCORES, NBATCH * 128, 3 * NCOL)
    tr = ip.reshape(NCORES, NBATCH, NCOL, 128).transpose(0, 1, 3, 2)
    tr = np.ascontiguousarray(tr).reshape(NCORES, NBATCH * 128, NCOL)
    ftab = np.ascontiguousarray(np.asarray(params, dtype=np.float32).reshape(N_TREES * L * T, F))
    return xr, tr, ftab, n


def kernel(block_x, params, block_inds):
    from concourse.bass_utils import run_bass_kernel_spmd

    xr, tr, ftab, n = _prep(block_x, params, block_inds)
    if "nc" not in _CACHE:
        _CACHE["nc"] = _build_nc()
    nc = _CACHE["nc"]
    in_maps = [{"xb": xr[c], "tb": tr[c], "ftab": ftab} for c in range(NCORES)]
    res = run_bass_kernel_spmd(nc, in_maps, core_ids=list(range(NCORES)))
    outs = [res.results[c]["out"] for c in range(NCORES)]
    full = np.concatenate(outs, axis=0)[:n]
    return np.ascontiguousarray(full)


# revision 9
# speedup vs baseline: 1.7186x; 1.6337x over previous
"""LoTD forest encoding (NGP-style multi-level hash grid, 8-tree forest) on TRN2.

Data-parallel over points across 8 NeuronCores. Per core, a hardware loop
(Tile For_i) processes batches of 1024 points:
  - DVE computes the spatial-hash table rows for all 8 trilinear corners of
    each point at each of the 16 levels (fp32-exact arithmetic for the
    mod-2^17 multiplies, int32 xor/and for the hash combine),
  - the per-corner feature pairs are fetched from a replicated bf16 copy of
    the full [8 trees x 16 levels x 131072 x 2] table in device DRAM via
    SWDGE indirect DMA (128 offsets per instruction, one per partition),
  - DVE applies the trilinear corner weights and accumulates the [N, 32]
    output, streamed back per batch.

The hash h = (x ^ y*P1 ^ z*P2) & (2^17-1) is computed without 32-bit integer
multiplies: only P mod 2^17 matters, and y*(P mod 2^17) is split into
(y%32)*(P mod 2^17) + (y//32)*((32*P) mod 2^17), both exact in fp32.

float->int casts on the vector engine round to nearest, so floor(pos) is
computed as cast(pos - 0.5), with the fractional weight w = pos - float(ip);
an off-by-one at exact-integer pos yields w=1.0 and the identical
interpolation result.
"""

import numpy as np

L = 16
F = 2
T = 1 << 17
N_TREES = 8
N = 2_000_000
RES = np.array([16, 22, 30, 41, 55, 75, 102, 139, 188, 256, 348, 472,
                642, 872, 1184, 1608], dtype=np.int64)
P1 = 2654435761
P2 = 805459861
MASK = T - 1
K1 = P1 & MASK            # y multiplier mod 2^17
K1H = (32 * P1) & MASK
K2 = P2 & MASK            # z multiplier mod 2^17
K2H = (32 * P2) & MASK

NCORES = 8
B = 1024                  # points per batch
NCOL = B // 128           # 8 point-columns per partition
NBATCH = 245              # batches per core
NPC = NBATCH * B          # 250880 points per core (padded)

_CACHE = {}


def _build_nc():
    import concourse.bass as bass
    import concourse.bacc as bacc
    import concourse.mybir as mybir
    import concourse.tile as tile
    from concourse.bass import IndirectOffsetOnAxis

    fp32 = mybir.dt.float32
    int32 = mybir.dt.int32
    bf16 = mybir.dt.bfloat16
    AO = mybir.AluOpType

    nc = bacc.Bacc("TRN2")
    # xb[b*128+p, d*NCOL+j] = x of point (b, j*128+p), dim d
    xb = nc.dram_tensor("xb", [NBATCH * 128, 3 * NCOL], fp32, kind="ExternalInput")
    # tb[b*128+p, j] = tree id (int32)
    tb = nc.dram_tensor("tb", [NBATCH * 128, NCOL], int32, kind="ExternalInput")
    # ftab[(tree*16+l)*T + h] = feature pair (bf16)
    ftab = nc.dram_tensor("ftab", [N_TREES * L * T, F], bf16, kind="ExternalInput")
    out = nc.dram_tensor("out", [NPC, 2 * L], fp32, kind="ExternalOutput")

    with tile.TileContext(nc) as tc:
        with tc.tile_pool(name="sbuf", bufs=1) as pool:
            x_t = pool.tile([128, 3 * NCOL], fp32, tag="x")
            t_i = pool.tile([128, NCOL], int32, tag="ti")
            tbase = pool.tile([128, NCOL], int32, tag="tbase")
            pos = pool.tile([128, 3 * NCOL], fp32, tag="pos")
            ipi = pool.tile([128, 3 * NCOL], int32, tag="ipi")
            ipf = pool.tile([128, 3 * NCOL], fp32, tag="ipf")
            w3 = pool.tile([128, 3 * NCOL], fp32, tag="w3")
            w3m = pool.tile([128, 3 * NCOL], fp32, tag="w3m")     # 1 - w
            hh = pool.tile([128, NCOL], fp32, tag="hh")           # scratch f
            hl = pool.tile([128, NCOL], fp32, tag="hl")
            tyf = pool.tile([128, 2 * NCOL], fp32, tag="tyf")     # y,z terms f
            tyi = pool.tile([128, 2 * NCOL], int32, tag="tyi")    # y0,z0 int
            tyi1 = pool.tile([128, 2 * NCOL], int32, tag="tyi1")  # y1,z1 int
            ix1 = pool.tile([128, NCOL], int32, tag="ix1")
            xy = pool.tile([128, 4 * NCOL], int32, tag="xy")      # x^y for 4 combos
            hc = pool.tile([128, NCOL], int32, tag="hc")          # corner hash scratch
            idx_t = pool.tile([128, 8 * NCOL], int32, tag="idx")
            g = pool.tile([128, 16 * NCOL], bf16, tag="g")
            gf = pool.tile([128, 16 * NCOL], fp32, tag="gf")
            wyz = pool.tile([128, 4 * NCOL], fp32, tag="wyz")
            w8 = pool.tile([128, 8 * NCOL], fp32, tag="w8")
            w16 = pool.tile([128, 16 * NCOL], fp32, tag="w16")
            tsum = pool.tile([128, 2 * NCOL], fp32, tag="tsum")
            acc = pool.tile([128, NCOL, 2 * L], fp32, tag="acc")

            with tc.For_i(0, NBATCH) as bi:
                nc.sync.dma_start(out=x_t[:], in_=xb[bass.ts(bi, 128), :])
                nc.sync.dma_start(out=t_i[:], in_=tb[bass.ts(bi, 128), :])
                # tbase = tree * (16*T)
                nc.vector.tensor_scalar(out=tbase[:], in0=t_i[:], scalar1=L * T,
                                        scalar2=0, op0=AO.mult, op1=AO.add)
                for l in range(L):
                    R = int(RES[l])
                    s = (R - 1) * 0.5
                    # pos = x*s + s ; ip = round(pos-0.5) ; w = pos - ip
                    nc.vector.tensor_scalar(out=pos[:], in0=x_t[:], scalar1=s,
                                            scalar2=s, op0=AO.mult, op1=AO.add)
                    nc.vector.tensor_scalar(out=ipf[:], in0=pos[:], scalar1=1.0,
                                            scalar2=-0.5, op0=AO.mult, op1=AO.add)
                    nc.vector.tensor_copy(out=ipi[:], in_=ipf[:])
                    nc.vector.tensor_copy(out=ipf[:], in_=ipi[:])
                    nc.vector.tensor_tensor(out=w3[:], in0=pos[:], in1=ipf[:],
                                            op=AO.subtract)
                    nc.vector.tensor_scalar(out=w3m[:], in0=w3[:], scalar1=-1.0,
                                            scalar2=1.0, op0=AO.mult, op1=AO.add)
                    # y/z hash terms: t = (i%32)*K + (i//32)*KH  (exact fp32)
                    for d, (KA, KB) in ((1, (K1, K1H)), (2, (K2, K2H))):
                        src = ipf[:, d * NCOL:(d + 1) * NCOL]
                        nc.vector.tensor_scalar(out=hh[:], in0=src, scalar1=0.03125,
                                                scalar2=-0.5, op0=AO.mult, op1=AO.add)
                        nc.vector.tensor_copy(out=hc[:], in_=hh[:])      # int floor
                        nc.vector.tensor_copy(out=hh[:], in_=hc[:])      # back to f
                        nc.vector.tensor_scalar(out=hl[:], in0=hh[:], scalar1=-32.0,
                                                scalar2=0.0, op0=AO.mult, op1=AO.add)
                        nc.vector.tensor_tensor(out=hl[:], in0=src, in1=hl[:],
                                                op=AO.add)               # i%32
                        nc.vector.tensor_scalar(out=hl[:], in0=hl[:], scalar1=float(KA),
                                                scalar2=0.0, op0=AO.mult, op1=AO.add)
                        nc.vector.tensor_scalar(out=hh[:], in0=hh[:], scalar1=float(KB),
                                                scalar2=0.0, op0=AO.mult, op1=AO.add)
                        dst = tyf[:, (d - 1) * NCOL:d * NCOL]
                        nc.vector.tensor_tensor(out=dst, in0=hl[:], in1=hh[:], op=AO.add)
                    nc.vector.tensor_copy(out=tyi[:], in_=tyf[:])
                    # +K for the +1 corners
                    nc.vector.tensor_scalar(out=tyi1[:, :NCOL], in0=tyi[:, :NCOL],
                                            scalar1=K1, scalar2=0, op0=AO.add, op1=AO.add)
                    nc.vector.tensor_scalar(out=tyi1[:, NCOL:], in0=tyi[:, NCOL:],
                                            scalar1=K2, scalar2=0, op0=AO.add, op1=AO.add)
                    nc.vector.tensor_scalar(out=ix1[:], in0=ipi[:, :NCOL], scalar1=1,
                                            scalar2=0, op0=AO.add, op1=AO.add)
                    # xy[dx*2+dy] = ix_dx ^ ty_dy
                    for dx, xsrc in ((0, ipi[:, :NCOL]), (1, ix1[:])):
                        for dy, ysrc in ((0, tyi[:, :NCOL]), (1, tyi1[:, :NCOL])):
                            nc.vector.tensor_tensor(
                                out=xy[:, (dx * 2 + dy) * NCOL:(dx * 2 + dy + 1) * NCOL],
                                in0=xsrc, in1=ysrc, op=AO.bitwise_xor)
                    # corners c = dx*4 + dy*2 + dz (matches OFFS ordering)
                    for dx in range(2):
                        for dy in range(2):
                            for dz in range(2):
                                c = dx * 4 + dy * 2 + dz
                                zsrc = tyi[:, NCOL:] if dz == 0 else tyi1[:, NCOL:]
                                nc.vector.tensor_tensor(
                                    out=hc[:],
                                    in0=xy[:, (dx * 2 + dy) * NCOL:(dx * 2 + dy + 1) * NCOL],
                                    in1=zsrc, op=AO.bitwise_xor)
                                nc.vector.tensor_scalar(out=hc[:], in0=hc[:],
                                                        scalar1=MASK, scalar2=l * T,
                                                        op0=AO.bitwise_and, op1=AO.bitwise_or)
                                nc.vector.tensor_tensor(
                                    out=idx_t[:, c * NCOL:(c + 1) * NCOL],
                                    in0=hc[:], in1=tbase[:], op=AO.add)
                    # gathers: one 128-offset indirect DMA per (corner, column)
                    for q in range(8 * NCOL):
                        nc.gpsimd.indirect_dma_start(
                            out=g[:, 2 * q:2 * q + 2],
                            out_offset=None,
                            in_=ftab[:],
                            in_offset=IndirectOffsetOnAxis(ap=idx_t[:, q:q + 1], axis=0),
                        )
                    # weights: w8[c] = wx_dx * wy_dy * wz_dz
                    for dy in range(2):
                        ws = w3m if dy == 0 else w3
                        for dz in range(2):
                            zs = w3m if dz == 0 else w3
                            nc.vector.tensor_tensor(
                                out=wyz[:, (dy * 2 + dz) * NCOL:(dy * 2 + dz + 1) * NCOL],
                                in0=ws[:, NCOL:2 * NCOL], in1=zs[:, 2 * NCOL:3 * NCOL],
                                op=AO.mult)
                    for dx in range(2):
                        xs = w3m if dx == 0 else w3
                        for k in range(4):
                            c = dx * 4 + k
                            nc.vector.tensor_tensor(
                                out=w8[:, c * NCOL:(c + 1) * NCOL],
                                in0=xs[:, :NCOL], in1=wyz[:, k * NCOL:(k + 1) * NCOL],
                                op=AO.mult)
                    # duplicate each weight across the 2 features
                    w16v = w16[:].rearrange("p (q two) -> p q two", two=2)
                    w8v = w8[:].rearrange("p (q one) -> p q one", one=1)
                    nc.vector.tensor_copy(out=w16v[:, :, 0:1], in_=w8v)
                    nc.vector.tensor_copy(out=w16v[:, :, 1:2], in_=w8v)
                    # weighted sum over corners
                    nc.vector.tensor_copy(out=gf[:], in_=g[:])
                    nc.vector.tensor_tensor(out=gf[:], in0=gf[:], in1=w16[:], op=AO.mult)
                    nc.vector.tensor_tensor(out=tsum[:], in0=gf[:, :2 * NCOL],
                                            in1=gf[:, 2 * NCOL:4 * NCOL], op=AO.add)
                    for c in range(2, 8):
                        nc.vector.tensor_tensor(
                            out=tsum[:], in0=tsum[:],
                            in1=gf[:, c * 2 * NCOL:(c + 1) * 2 * NCOL], op=AO.add)
                    # tsum[p, j*2+f] -> acc[p, j, 2l+f]
                    nc.vector.tensor_copy(
                        out=acc[:, :, 2 * l:2 * l + 2],
                        in_=tsum[:].rearrange("p (j f) -> p j f", f=2))
                # out rows j*128+p <- acc[p, j, :]
                ov = out[bass.ts(bi, B), :].rearrange("(j p) f -> p j f", p=128)
                nc.sync.dma_start(out=ov, in_=acc[:])
    nc.compile()
    return nc


def _prep(block_x, params, block_inds):
    x = np.asarray(block_x, dtype=np.float32)
    inds = np.asarray(block_inds).astype(np.int32)
    n = x.shape[0]
    ntot = NCORES * NPC
    xp = np.zeros((ntot, 3), dtype=np.float32)
    xp[:n] = x
    ip = np.zeros(ntot, dtype=np.int32)
    ip[:n] = inds
    # [c, b, j, p, d] -> [c, b*128+p, d*NCOL+j]
    xr = xp.reshape(NCORES, NBATCH, NCOL, 128, 3).transpose(0, 1, 3, 4, 2)
    xr = np.ascontiguousarray(xr).reshape(NCORES, NBATCH * 128, 3 * NCOL)
    tr = ip.reshape(NCORES, NBATCH, NCOL, 128).transpose(0, 1, 3, 2)
    tr = np.ascontiguousarray(tr).reshape(NCORES, NBATCH * 128, NCOL)
    import ml_dtypes
    ftab = np.asarray(params, dtype=np.float32).reshape(N_TREES * L * T, F).astype(ml_dtypes.bfloat16)
    return xr, tr, ftab, n


def kernel(block_x, params, block_inds):
    import time as _t
    from concourse.bass_utils import run_bass_kernel_spmd

    _t0 = _t.time()
    xr, tr, ftab, n = _prep(block_x, params, block_inds)
    _t1 = _t.time()
    if "nc" not in _CACHE:
        _CACHE["nc"] = _build_nc()
    nc = _CACHE["nc"]
    in_maps = [{"xb": xr[c], "tb": tr[c], "ftab": ftab} for c in range(NCORES)]
    _t2 = _t.time()
    res = run_bass_kernel_spmd(nc, in_maps, core_ids=list(range(NCORES)))
    _t3 = _t.time()
    outs = [res.results[c]["out"] for c in range(NCORES)]
    import os
    if os.environ.get("KERNEL_VERBOSE"):
        print(f"[kernel] prep={_t1-_t0:.2f}s build={_t2-_t1:.2f}s spmd={_t3-_t2:.2f}s")
    full = np.concatenate(outs, axis=0)[:n]
    return np.ascontiguousarray(full)


# revision 10
# speedup vs baseline: 2.6809x; 1.5599x over previous
"""LoTD forest encoding (NGP-style multi-level hash grid, 8-tree forest) on TRN2.

Data-parallel over points across 8 NeuronCores. Per core, a hardware loop
(Tile For_i) processes batches of 1024 points:
  - DVE computes the spatial-hash table rows for all 8 trilinear corners of
    each point at each of the 16 levels (fp32-exact arithmetic for the
    mod-2^17 multiplies, int32 xor/and for the hash combine),
  - the per-corner feature pairs are fetched from a replicated bf16 copy of
    the full [8 trees x 16 levels x 131072 x 2] table in device DRAM via
    SWDGE indirect DMA (128 offsets per instruction, one per partition),
  - DVE applies the trilinear corner weights and accumulates the [N, 32]
    output, streamed back per batch.

The hash h = (x ^ y*P1 ^ z*P2) & (2^17-1) is computed without 32-bit integer
multiplies: only P mod 2^17 matters, and y*(P mod 2^17) is split into
(y%32)*(P mod 2^17) + (y//32)*((32*P) mod 2^17), both exact in fp32.

float->int casts on the vector engine round to nearest, so floor(pos) is
computed as cast(pos - 0.5), with the fractional weight w = pos - float(ip);
an off-by-one at exact-integer pos yields w=1.0 and the identical
interpolation result.
"""

import numpy as np

L = 16
F = 2
T = 1 << 17
N_TREES = 8
N = 2_000_000
RES = np.array([16, 22, 30, 41, 55, 75, 102, 139, 188, 256, 348, 472,
                642, 872, 1184, 1608], dtype=np.int64)
P1 = 2654435761
P2 = 805459861
MASK = T - 1
K1 = P1 & MASK            # y multiplier mod 2^17
K1H = (32 * P1) & MASK
K2 = P2 & MASK            # z multiplier mod 2^17
K2H = (32 * P2) & MASK

NCORES = 8
B = 1024                  # points per batch
NCOL = B // 128           # 8 point-columns per partition
NBATCH = 245              # batches per core
NPC = NBATCH * B          # 250880 points per core (padded)

_CACHE = {}


def _build_nc():
    import concourse.bass as bass
    import concourse.bacc as bacc
    import concourse.mybir as mybir
    import concourse.tile as tile
    from concourse.bass import IndirectOffsetOnAxis

    fp32 = mybir.dt.float32
    int32 = mybir.dt.int32
    bf16 = mybir.dt.bfloat16
    AO = mybir.AluOpType

    nc = bacc.Bacc("TRN2")
    # xb[b*128+p, d*NCOL+j] = x of point (b, j*128+p), dim d
    xb = nc.dram_tensor("xb", [NBATCH * 128, 3 * NCOL], fp32, kind="ExternalInput")
    # ftab[l*T + h] = feature pair for this core's tree (bf16)
    ftab = nc.dram_tensor("ftab", [L * T, F], bf16, kind="ExternalInput")
    out = nc.dram_tensor("out", [NPC, 2 * L], bf16, kind="ExternalOutput")

    with tile.TileContext(nc) as tc:
        with tc.tile_pool(name="sbuf", bufs=1) as pool:
            x_t = pool.tile([128, 3 * NCOL], fp32, tag="x")
            pos = pool.tile([128, 3 * NCOL], fp32, tag="pos")
            ipi = pool.tile([128, 3 * NCOL], int32, tag="ipi")
            ipf = pool.tile([128, 3 * NCOL], fp32, tag="ipf")
            w3 = pool.tile([128, 3 * NCOL], fp32, tag="w3")
            w3m = pool.tile([128, 3 * NCOL], fp32, tag="w3m")     # 1 - w
            hh = pool.tile([128, NCOL], fp32, tag="hh")           # scratch f
            hl = pool.tile([128, NCOL], fp32, tag="hl")
            tyf = pool.tile([128, 2 * NCOL], fp32, tag="tyf")     # y,z terms f
            tyi = pool.tile([128, 2 * NCOL], int32, tag="tyi")    # y0,z0 int
            tyi1 = pool.tile([128, 2 * NCOL], int32, tag="tyi1")  # y1,z1 int
            ix1 = pool.tile([128, NCOL], int32, tag="ix1")
            xy = pool.tile([128, 4 * NCOL], int32, tag="xy")      # x^y for 4 combos
            hc = pool.tile([128, NCOL], int32, tag="hc")          # corner hash scratch
            idx_t = pool.tile([128, 8 * NCOL], int32, tag="idx")
            g = pool.tile([128, 16 * NCOL], bf16, tag="g")
            gf = pool.tile([128, 16 * NCOL], fp32, tag="gf")
            wyz = pool.tile([128, 4 * NCOL], fp32, tag="wyz")
            w8 = pool.tile([128, 8 * NCOL], fp32, tag="w8")
            w16 = pool.tile([128, 16 * NCOL], fp32, tag="w16")
            tsum = pool.tile([128, 2 * NCOL], fp32, tag="tsum")
            acc = pool.tile([128, NCOL, 2 * L], fp32, tag="acc")
            accb = pool.tile([128, NCOL, 2 * L], bf16, tag="accb")

            with tc.For_i(0, NBATCH) as bi:
                nc.sync.dma_start(out=x_t[:], in_=xb[bass.ts(bi, 128), :])
                for l in range(L):
                    R = int(RES[l])
                    s = (R - 1) * 0.5
                    # pos = x*s + s ; ip = round(pos-0.5) ; w = pos - ip
                    nc.vector.tensor_scalar(out=pos[:], in0=x_t[:], scalar1=s,
                                            scalar2=s, op0=AO.mult, op1=AO.add)
                    nc.vector.tensor_scalar(out=ipf[:], in0=pos[:], scalar1=1.0,
                                            scalar2=-0.5, op0=AO.mult, op1=AO.add)
                    nc.vector.tensor_copy(out=ipi[:], in_=ipf[:])
                    nc.vector.tensor_copy(out=ipf[:], in_=ipi[:])
                    nc.vector.tensor_tensor(out=w3[:], in0=pos[:], in1=ipf[:],
                                            op=AO.subtract)
                    nc.vector.tensor_scalar(out=w3m[:], in0=w3[:], scalar1=-1.0,
                                            scalar2=1.0, op0=AO.mult, op1=AO.add)
                    # y/z hash terms: t = (i%32)*K + (i//32)*KH  (exact fp32)
                    for d, (KA, KB) in ((1, (K1, K1H)), (2, (K2, K2H))):
                        src = ipf[:, d * NCOL:(d + 1) * NCOL]
                        nc.vector.tensor_scalar(out=hh[:], in0=src, scalar1=0.03125,
                                                scalar2=-0.5, op0=AO.mult, op1=AO.add)
                        nc.vector.tensor_copy(out=hc[:], in_=hh[:])      # int floor
                        nc.vector.tensor_copy(out=hh[:], in_=hc[:])      # back to f
                        nc.vector.tensor_scalar(out=hl[:], in0=hh[:], scalar1=-32.0,
                                                scalar2=0.0, op0=AO.mult, op1=AO.add)
                        nc.vector.tensor_tensor(out=hl[:], in0=src, in1=hl[:],
                                                op=AO.add)               # i%32
                        nc.vector.tensor_scalar(out=hl[:], in0=hl[:], scalar1=float(KA),
                                                scalar2=0.0, op0=AO.mult, op1=AO.add)
                        nc.vector.tensor_scalar(out=hh[:], in0=hh[:], scalar1=float(KB),
                                                scalar2=0.0, op0=AO.mult, op1=AO.add)
                        dst = tyf[:, (d - 1) * NCOL:d * NCOL]
                        nc.vector.tensor_tensor(out=dst, in0=hl[:], in1=hh[:], op=AO.add)
                    nc.vector.tensor_copy(out=tyi[:], in_=tyf[:])
                    # +K for the +1 corners
                    nc.vector.tensor_scalar(out=tyi1[:, :NCOL], in0=tyi[:, :NCOL],
                                            scalar1=K1, scalar2=0, op0=AO.add, op1=AO.add)
                    nc.vector.tensor_scalar(out=tyi1[:, NCOL:], in0=tyi[:, NCOL:],
                                            scalar1=K2, scalar2=0, op0=AO.add, op1=AO.add)
                    nc.vector.tensor_scalar(out=ix1[:], in0=ipi[:, :NCOL], scalar1=1,
                                            scalar2=0, op0=AO.add, op1=AO.add)
                    # xy[dx*2+dy] = ix_dx ^ ty_dy
                    for dx, xsrc in ((0, ipi[:, :NCOL]), (1, ix1[:])):
                        for dy, ysrc in ((0, tyi[:, :NCOL]), (1, tyi1[:, :NCOL])):
                            nc.vector.tensor_tensor(
                                out=xy[:, (dx * 2 + dy) * NCOL:(dx * 2 + dy + 1) * NCOL],
                                in0=xsrc, in1=ysrc, op=AO.bitwise_xor)
                    # corners c = dx*4 + dy*2 + dz (matches OFFS ordering)
                    for dx in range(2):
                        for dy in range(2):
                            for dz in range(2):
                                c = dx * 4 + dy * 2 + dz
                                zsrc = tyi[:, NCOL:] if dz == 0 else tyi1[:, NCOL:]
                                nc.vector.tensor_tensor(
                                    out=hc[:],
                                    in0=xy[:, (dx * 2 + dy) * NCOL:(dx * 2 + dy + 1) * NCOL],
                                    in1=zsrc, op=AO.bitwise_xor)
                                nc.vector.tensor_scalar(out=idx_t[:, c * NCOL:(c + 1) * NCOL],
                                                        in0=hc[:],
                                                        scalar1=MASK, scalar2=l * T,
                                                        op0=AO.bitwise_and, op1=AO.bitwise_or)
                    # gathers: one 128-offset indirect DMA per (corner, column)
                    for q in range(8 * NCOL):
                        nc.gpsimd.indirect_dma_start(
                            out=g[:, 2 * q:2 * q + 2],
                            out_offset=None,
                            in_=ftab[:],
                            in_offset=IndirectOffsetOnAxis(ap=idx_t[:, q:q + 1], axis=0),
                        )
                    # weights: w8[c] = wx_dx * wy_dy * wz_dz
                    for dy in range(2):
                        ws = w3m if dy == 0 else w3
                        for dz in range(2):
                            zs = w3m if dz == 0 else w3
                            nc.vector.tensor_tensor(
                                out=wyz[:, (dy * 2 + dz) * NCOL:(dy * 2 + dz + 1) * NCOL],
                                in0=ws[:, NCOL:2 * NCOL], in1=zs[:, 2 * NCOL:3 * NCOL],
                                op=AO.mult)
                    for dx in range(2):
                        xs = w3m if dx == 0 else w3
                        for k in range(4):
                            c = dx * 4 + k
                            nc.vector.tensor_tensor(
                                out=w8[:, c * NCOL:(c + 1) * NCOL],
                                in0=xs[:, :NCOL], in1=wyz[:, k * NCOL:(k + 1) * NCOL],
                                op=AO.mult)
                    # duplicate each weight across the 2 features
                    w16v = w16[:].rearrange("p (q two) -> p q two", two=2)
                    w8v = w8[:].rearrange("p (q one) -> p q one", one=1)
                    nc.vector.tensor_copy(out=w16v[:, :, 0:1], in_=w8v)
                    nc.vector.tensor_copy(out=w16v[:, :, 1:2], in_=w8v)
                    # weighted sum over corners
                    nc.vector.tensor_copy(out=gf[:], in_=g[:])
                    nc.vector.tensor_tensor(out=gf[:], in0=gf[:], in1=w16[:], op=AO.mult)
                    nc.vector.tensor_tensor(out=tsum[:], in0=gf[:, :2 * NCOL],
                                            in1=gf[:, 2 * NCOL:4 * NCOL], op=AO.add)
                    for c in range(2, 8):
                        nc.vector.tensor_tensor(
                            out=tsum[:], in0=tsum[:],
                            in1=gf[:, c * 2 * NCOL:(c + 1) * 2 * NCOL], op=AO.add)
                    # tsum[p, j*2+f] -> acc[p, j, 2l+f]
                    nc.vector.tensor_copy(
                        out=acc[:, :, 2 * l:2 * l + 2],
                        in_=tsum[:].rearrange("p (j f) -> p j f", f=2))
                # out rows j*128+p <- acc[p, j, :]
                nc.vector.tensor_copy(out=accb[:], in_=acc[:])
                ov = out[bass.ts(bi, B), :].rearrange("(j p) f -> p j f", p=128)
                nc.sync.dma_start(out=ov, in_=accb[:])
    nc.compile()
    return nc


def _prep(block_x, params, block_inds):
    import ml_dtypes
    x = np.asarray(block_x, dtype=np.float32)
    inds = np.asarray(block_inds).astype(np.int64)
    n = x.shape[0]
    order = np.argsort(inds, kind="stable")
    counts = np.bincount(inds, minlength=N_TREES)
    starts = np.concatenate([[0], np.cumsum(counts)])
    bucket_idx = []   # global point ids handled by core c, in device row order
    xr = np.zeros((NCORES, NBATCH * 128, 3 * NCOL), dtype=np.float32)
    overflow = []     # (global ids) handled on host (bucket overflow; ~never)
    for c in range(NCORES):
        ids = order[starts[c]:starts[c + 1]]
        if len(ids) > NPC:
            overflow.append(ids[NPC:])
            ids = ids[:NPC]
        bucket_idx.append(ids)
        xc = np.zeros((NPC, 3), dtype=np.float32)
        xc[:len(ids)] = x[ids]
        xr[c] = np.ascontiguousarray(
            xc.reshape(NBATCH, NCOL, 128, 3).transpose(0, 2, 3, 1)
        ).reshape(NBATCH * 128, 3 * NCOL)
    ftabs = np.asarray(params, dtype=np.float32).reshape(
        N_TREES, L * T, F).astype(ml_dtypes.bfloat16)
    return xr, ftabs, bucket_idx, overflow, n


def _host_ref(block_x, params, ids, inds):
    """Exact numpy fallback for overflow points (rare)."""
    OFFS = np.stack(np.meshgrid([0, 1], [0, 1], [0, 1], indexing="ij"),
                    axis=-1).reshape(8, 3).astype(np.int32)
    x01 = np.asarray(block_x, np.float32)[ids] * np.float32(0.5) + np.float32(0.5)
    t = np.asarray(inds)[ids].astype(np.int64)
    out = np.empty((len(ids), 2 * L), dtype=np.float32)
    offs_b = OFFS.astype(bool)
    for l in range(L):
        R = int(RES[l])
        pos = x01 * np.float32(R - 1)
        p0 = np.floor(pos)
        w = pos - p0
        p0i = p0.astype(np.int32)
        corners = np.clip(p0i[:, None, :] + OFFS[None], 0, R - 1)
        cu = corners.astype(np.uint32)
        h = (cu[..., 0] ^ (cu[..., 1] * np.uint32(P1 & 0xFFFFFFFF))
             ^ (cu[..., 2] * np.uint32(P2 & 0xFFFFFFFF)))
        idx = (h & np.uint32(MASK)).astype(np.int64)
        feats = np.asarray(params, np.float32)[t[:, None], l, idx, :]
        wc = np.prod(np.where(offs_b[None], w[:, None, :],
                              np.float32(1.0) - w[:, None, :]), axis=-1)
        out[:, 2 * l:2 * l + 2] = np.einsum("nc,ncf->nf",
                                            wc.astype(np.float32), feats)
    return out


def kernel(block_x, params, block_inds):
    import time as _t
    from concourse.bass_utils import run_bass_kernel_spmd

    _t0 = _t.time()
    xr, ftabs, bucket_idx, overflow, n = _prep(block_x, params, block_inds)
    _t1 = _t.time()
    if "nc" not in _CACHE:
        _CACHE["nc"] = _build_nc()
    nc = _CACHE["nc"]
    in_maps = [{"xb": xr[c], "ftab": np.ascontiguousarray(ftabs[c])}
               for c in range(NCORES)]
    _t2 = _t.time()
    res = run_bass_kernel_spmd(nc, in_maps, core_ids=list(range(NCORES)))
    _t3 = _t.time()
    full = np.empty((n, 2 * L), dtype=np.float32)
    for c in range(NCORES):
        ids = bucket_idx[c]
        full[ids] = res.results[c]["out"][:len(ids)].astype(np.float32)
    if overflow:
        ids = np.concatenate(overflow)
        full[ids] = _host_ref(block_x, params, ids, block_inds)
    import os
    if os.environ.get("KERNEL_VERBOSE"):
        print(f"[kernel] prep={_t1-_t0:.2f}s build={_t2-_t1:.2f}s "
              f"spmd={_t3-_t2:.2f}s post={_t.time()-_t3:.2f}s")
    return full


# revision 11
# speedup vs baseline: 4.3502x; 1.6227x over previous
"""LoTD forest encoding (NGP-style multi-level hash grid, 8-tree forest) on TRN2.

Data-parallel over points across 8 NeuronCores. Per core, a hardware loop
(Tile For_i) processes batches of 1024 points:
  - DVE computes the spatial-hash table rows for all 8 trilinear corners of
    each point at each of the 16 levels (fp32-exact arithmetic for the
    mod-2^17 multiplies, int32 xor/and for the hash combine),
  - the per-corner feature pairs are fetched from a replicated bf16 copy of
    the full [8 trees x 16 levels x 131072 x 2] table in device DRAM via
    SWDGE indirect DMA (128 offsets per instruction, one per partition),
  - DVE applies the trilinear corner weights and accumulates the [N, 32]
    output, streamed back per batch.

The hash h = (x ^ y*P1 ^ z*P2) & (2^17-1) is computed without 32-bit integer
multiplies: only P mod 2^17 matters, and y*(P mod 2^17) is split into
(y%32)*(P mod 2^17) + (y//32)*((32*P) mod 2^17), both exact in fp32.

float->int casts on the vector engine round to nearest, so floor(pos) is
computed as cast(pos - 0.5), with the fractional weight w = pos - float(ip);
an off-by-one at exact-integer pos yields w=1.0 and the identical
interpolation result.
"""

import numpy as np

L = 16
F = 2
T = 1 << 17
N_TREES = 8
N = 2_000_000
RES = np.array([16, 22, 30, 41, 55, 75, 102, 139, 188, 256, 348, 472,
                642, 872, 1184, 1608], dtype=np.int64)
P1 = 2654435761
P2 = 805459861
MASK = T - 1
K1 = P1 & MASK            # y multiplier mod 2^17
K1H = (32 * P1) & MASK
K2 = P2 & MASK            # z multiplier mod 2^17
K2H = (32 * P2) & MASK

NCORES = 8
B = 1024                  # points per batch
NCOL = B // 128           # 8 point-columns per partition
NBATCH = 245              # batches per core
NPC = NBATCH * B          # 250880 points per core (padded)

_CACHE = {}


def _build_nc():
    import concourse.bass as bass
    import concourse.bacc as bacc
    import concourse.mybir as mybir
    import concourse.tile as tile
    from concourse.bass import IndirectOffsetOnAxis

    fp32 = mybir.dt.float32
    int32 = mybir.dt.int32
    bf16 = mybir.dt.bfloat16
    AO = mybir.AluOpType

    nc = bacc.Bacc("TRN2")
    # xb[b*128+p, d*NCOL+j] = x of point (b, j*128+p), dim d
    xb = nc.dram_tensor("xb", [NBATCH * 128, 3 * NCOL], fp32, kind="ExternalInput")
    # ftab[l*T + h] = feature pair for this core's tree (bf16)
    ftab = nc.dram_tensor("ftab", [L * T, F], bf16, kind="ExternalInput")
    out = nc.dram_tensor("out", [NPC, 2 * L], bf16, kind="ExternalOutput")

    with tile.TileContext(nc) as tc:
        with tc.tile_pool(name="sbuf", bufs=1) as pool:
            x_t = pool.tile([128, 3 * NCOL], fp32, tag="x")
            pos = pool.tile([128, 3 * NCOL], fp32, tag="pos")
            ipi = pool.tile([128, 3 * NCOL], int32, tag="ipi")
            ipf = pool.tile([128, 3 * NCOL], fp32, tag="ipf")
            w3 = pool.tile([128, 3 * NCOL], fp32, tag="w3")
            w3m = pool.tile([128, 3 * NCOL], fp32, tag="w3m")     # 1 - w
            hh = pool.tile([128, NCOL], fp32, tag="hh")           # scratch f
            hl = pool.tile([128, NCOL], fp32, tag="hl")
            tyf = pool.tile([128, 2 * NCOL], fp32, tag="tyf")     # y,z terms f
            tyi = pool.tile([128, 2 * NCOL], int32, tag="tyi")    # y0,z0 int
            tyi1 = pool.tile([128, 2 * NCOL], int32, tag="tyi1")  # y1,z1 int
            ix1 = pool.tile([128, NCOL], int32, tag="ix1")
            xy = pool.tile([128, 4 * NCOL], int32, tag="xy")      # x^y for 4 combos
            hc = pool.tile([128, NCOL], int32, tag="hc")          # corner hash scratch
            idx_t = pool.tile([128, 8 * NCOL], int32, tag="idx")
            g = pool.tile([128, 16 * NCOL], bf16, tag="g")
            gf = pool.tile([128, 16 * NCOL], fp32, tag="gf")
            wyz = pool.tile([128, 4 * NCOL], fp32, tag="wyz")
            w8 = pool.tile([128, 8 * NCOL], fp32, tag="w8")
            w16 = pool.tile([128, 16 * NCOL], fp32, tag="w16")
            tsum = pool.tile([128, 2 * NCOL], fp32, tag="tsum")
            acc = pool.tile([128, NCOL, 2 * L], fp32, tag="acc")
            accb = pool.tile([128, NCOL, 2 * L], bf16, tag="accb")

            with tc.For_i(0, NBATCH) as bi:
                nc.sync.dma_start(out=x_t[:], in_=xb[bass.ts(bi, 128), :])
                for l in range(L):
                    R = int(RES[l])
                    s = (R - 1) * 0.5
                    # pos = x*s + s ; ip = round(pos-0.5) ; w = pos - ip
                    nc.vector.tensor_scalar(out=pos[:], in0=x_t[:], scalar1=s,
                                            scalar2=s, op0=AO.mult, op1=AO.add)
                    nc.vector.tensor_scalar(out=ipf[:], in0=pos[:], scalar1=1.0,
                                            scalar2=-0.5, op0=AO.mult, op1=AO.add)
                    nc.vector.tensor_copy(out=ipi[:], in_=ipf[:])
                    nc.vector.tensor_copy(out=ipf[:], in_=ipi[:])
                    nc.vector.tensor_tensor(out=w3[:], in0=pos[:], in1=ipf[:],
                                            op=AO.subtract)
                    nc.vector.tensor_scalar(out=w3m[:], in0=w3[:], scalar1=-1.0,
                                            scalar2=1.0, op0=AO.mult, op1=AO.add)
                    # y/z hash terms: t = (i%32)*K + (i//32)*KH  (exact fp32)
                    for d, (KA, KB) in ((1, (K1, K1H)), (2, (K2, K2H))):
                        src = ipf[:, d * NCOL:(d + 1) * NCOL]
                        nc.vector.tensor_scalar(out=hh[:], in0=src, scalar1=0.03125,
                                                scalar2=-0.5, op0=AO.mult, op1=AO.add)
                        nc.vector.tensor_copy(out=hc[:], in_=hh[:])      # int floor
                        nc.vector.tensor_copy(out=hh[:], in_=hc[:])      # back to f
                        nc.vector.tensor_scalar(out=hl[:], in0=hh[:], scalar1=-32.0,
                                                scalar2=0.0, op0=AO.mult, op1=AO.add)
                        nc.vector.tensor_tensor(out=hl[:], in0=src, in1=hl[:],
                                                op=AO.add)               # i%32
                        nc.vector.tensor_scalar(out=hl[:], in0=hl[:], scalar1=float(KA),
                                                scalar2=0.0, op0=AO.mult, op1=AO.add)
                        nc.vector.tensor_scalar(out=hh[:], in0=hh[:], scalar1=float(KB),
                                                scalar2=0.0, op0=AO.mult, op1=AO.add)
                        dst = tyf[:, (d - 1) * NCOL:d * NCOL]
                        nc.vector.tensor_tensor(out=dst, in0=hl[:], in1=hh[:], op=AO.add)
                    nc.vector.tensor_copy(out=tyi[:], in_=tyf[:])
                    # +K for the +1 corners
                    nc.vector.tensor_scalar(out=tyi1[:, :NCOL], in0=tyi[:, :NCOL],
                                            scalar1=K1, scalar2=0, op0=AO.add, op1=AO.add)
                    nc.vector.tensor_scalar(out=tyi1[:, NCOL:], in0=tyi[:, NCOL:],
                                            scalar1=K2, scalar2=0, op0=AO.add, op1=AO.add)
                    nc.vector.tensor_scalar(out=ix1[:], in0=ipi[:, :NCOL], scalar1=1,
                                            scalar2=0, op0=AO.add, op1=AO.add)
                    # xy[dx*2+dy] = ix_dx ^ ty_dy
                    for dx, xsrc in ((0, ipi[:, :NCOL]), (1, ix1[:])):
                        for dy, ysrc in ((0, tyi[:, :NCOL]), (1, tyi1[:, :NCOL])):
                            nc.vector.tensor_tensor(
                                out=xy[:, (dx * 2 + dy) * NCOL:(dx * 2 + dy + 1) * NCOL],
                                in0=xsrc, in1=ysrc, op=AO.bitwise_xor)
                    # corners c = dx*4 + dy*2 + dz (matches OFFS ordering)
                    for dx in range(2):
                        for dy in range(2):
                            for dz in range(2):
                                c = dx * 4 + dy * 2 + dz
                                zsrc = tyi[:, NCOL:] if dz == 0 else tyi1[:, NCOL:]
                                nc.vector.tensor_tensor(
                                    out=hc[:],
                                    in0=xy[:, (dx * 2 + dy) * NCOL:(dx * 2 + dy + 1) * NCOL],
                                    in1=zsrc, op=AO.bitwise_xor)
                                nc.vector.tensor_scalar(out=idx_t[:, c * NCOL:(c + 1) * NCOL],
                                                        in0=hc[:],
                                                        scalar1=MASK, scalar2=l * T,
                                                        op0=AO.bitwise_and, op1=AO.bitwise_or)
                    # gathers: one 128-offset indirect DMA per (corner, column)
                    for q in range(8 * NCOL):
                        nc.gpsimd.indirect_dma_start(
                            out=g[:, 2 * q:2 * q + 2],
                            out_offset=None,
                            in_=ftab[:],
                            in_offset=IndirectOffsetOnAxis(ap=idx_t[:, q:q + 1], axis=0),
                        )
                    # weights: w8[c] = wx_dx * wy_dy * wz_dz
                    for dy in range(2):
                        ws = w3m if dy == 0 else w3
                        for dz in range(2):
                            zs = w3m if dz == 0 else w3
                            nc.vector.tensor_tensor(
                                out=wyz[:, (dy * 2 + dz) * NCOL:(dy * 2 + dz + 1) * NCOL],
                                in0=ws[:, NCOL:2 * NCOL], in1=zs[:, 2 * NCOL:3 * NCOL],
                                op=AO.mult)
                    for dx in range(2):
                        xs = w3m if dx == 0 else w3
                        for k in range(4):
                            c = dx * 4 + k
                            nc.vector.tensor_tensor(
                                out=w8[:, c * NCOL:(c + 1) * NCOL],
                                in0=xs[:, :NCOL], in1=wyz[:, k * NCOL:(k + 1) * NCOL],
                                op=AO.mult)
                    # duplicate each weight across the 2 features
                    w16v = w16[:].rearrange("p (q two) -> p q two", two=2)
                    w8v = w8[:].rearrange("p (q one) -> p q one", one=1)
                    nc.vector.tensor_copy(out=w16v[:, :, 0:1], in_=w8v)
                    nc.vector.tensor_copy(out=w16v[:, :, 1:2], in_=w8v)
                    # weighted sum over corners
                    nc.vector.tensor_copy(out=gf[:], in_=g[:])
                    nc.vector.tensor_tensor(out=gf[:], in0=gf[:], in1=w16[:], op=AO.mult)
                    nc.vector.tensor_tensor(out=tsum[:], in0=gf[:, :2 * NCOL],
                                            in1=gf[:, 2 * NCOL:4 * NCOL], op=AO.add)
                    for c in range(2, 8):
                        nc.vector.tensor_tensor(
                            out=tsum[:], in0=tsum[:],
                            in1=gf[:, c * 2 * NCOL:(c + 1) * 2 * NCOL], op=AO.add)
                    # tsum[p, j*2+f] -> acc[p, j, 2l+f]
                    nc.vector.tensor_copy(
                        out=acc[:, :, 2 * l:2 * l + 2],
                        in_=tsum[:].rearrange("p (j f) -> p j f", f=2))
                # out rows j*128+p <- acc[p, j, :]
                nc.vector.tensor_copy(out=accb[:], in_=acc[:])
                ov = out[bass.ts(bi, B), :].rearrange("(j p) f -> p j f", p=128)
                nc.sync.dma_start(out=ov, in_=accb[:])
    nc.compile()
    return nc


def _prep(block_x, params, block_inds):
    import ml_dtypes
    x = np.asarray(block_x, dtype=np.float32)
    inds = np.asarray(block_inds).astype(np.int64)
    n = x.shape[0]
    order = np.argsort(inds, kind="stable")
    counts = np.bincount(inds, minlength=N_TREES)
    starts = np.concatenate([[0], np.cumsum(counts)])
    bucket_idx = []   # global point ids handled by core c, in device row order
    xr = np.zeros((NCORES, NBATCH * 128, 3 * NCOL), dtype=np.float32)
    overflow = []     # (global ids) handled on host (bucket overflow; ~never)
    for c in range(NCORES):
        ids = order[starts[c]:starts[c + 1]]
        if len(ids) > NPC:
            overflow.append(ids[NPC:])
            ids = ids[:NPC]
        bucket_idx.append(ids)
        xc = np.zeros((NPC, 3), dtype=np.float32)
        xc[:len(ids)] = x[ids]
        xr[c] = np.ascontiguousarray(
            xc.reshape(NBATCH, NCOL, 128, 3).transpose(0, 2, 3, 1)
        ).reshape(NBATCH * 128, 3 * NCOL)
    ftabs = np.asarray(params, dtype=np.float32).reshape(
        N_TREES, L * T, F).astype(ml_dtypes.bfloat16)
    return xr, ftabs, bucket_idx, overflow, n


def _host_ref(block_x, params, ids, inds):
    """Exact numpy fallback for overflow points (rare)."""
    OFFS = np.stack(np.meshgrid([0, 1], [0, 1], [0, 1], indexing="ij"),
                    axis=-1).reshape(8, 3).astype(np.int32)
    x01 = np.asarray(block_x, np.float32)[ids] * np.float32(0.5) + np.float32(0.5)
    t = np.asarray(inds)[ids].astype(np.int64)
    out = np.empty((len(ids), 2 * L), dtype=np.float32)
    offs_b = OFFS.astype(bool)
    for l in range(L):
        R = int(RES[l])
        pos = x01 * np.float32(R - 1)
        p0 = np.floor(pos)
        w = pos - p0
        p0i = p0.astype(np.int32)
        corners = np.clip(p0i[:, None, :] + OFFS[None], 0, R - 1)
        cu = corners.astype(np.uint32)
        h = (cu[..., 0] ^ (cu[..., 1] * np.uint32(P1 & 0xFFFFFFFF))
             ^ (cu[..., 2] * np.uint32(P2 & 0xFFFFFFFF)))
        idx = (h & np.uint32(MASK)).astype(np.int64)
        feats = np.asarray(params, np.float32)[t[:, None], l, idx, :]
        wc = np.prod(np.where(offs_b[None], w[:, None, :],
                              np.float32(1.0) - w[:, None, :]), axis=-1)
        out[:, 2 * l:2 * l + 2] = np.einsum("nc,ncf->nf",
                                            wc.astype(np.float32), feats)
    return out


def kernel(block_x, params, block_inds):
    import time as _t
    from concourse.bass_utils import run_bass_kernel_spmd

    _t0 = _t.time()
    xr, ftabs, bucket_idx, overflow, n = _prep(block_x, params, block_inds)
    _t1 = _t.time()
    if "nc" not in _CACHE:
        _CACHE["nc"] = _build_nc()
    nc = _CACHE["nc"]
    in_maps = [{"xb": xr[c], "ftab": np.ascontiguousarray(ftabs[c])}
               for c in range(NCORES)]
    _t2 = _t.time()
    res = run_bass_kernel_spmd(nc, in_maps, core_ids=list(range(NCORES)))
    _t3 = _t.time()
    full = np.empty((n, 2 * L), dtype=np.float32)
    fu32 = full.view(np.uint32)
    for c in range(NCORES):
        ids = bucket_idx[c]
        raw = np.ascontiguousarray(res.results[c]["out"][:len(ids)])
        u = raw.view(np.uint16).astype(np.uint32)
        np.left_shift(u, 16, out=u)
        fu32[ids] = u
    if overflow:
        ids = np.concatenate(overflow)
        full[ids] = _host_ref(block_x, params, ids, block_inds)
    import os
    if os.environ.get("KERNEL_VERBOSE"):
        print(f"[kernel] prep={_t1-_t0:.2f}s build={_t2-_t1:.2f}s "
              f"spmd={_t3-_t2:.2f}s post={_t.time()-_t3:.2f}s")
    return full


# revision 12
# speedup vs baseline: 5.9909x; 1.3771x over previous
"""LoTD forest encoding (NGP-style multi-level hash grid, 8-tree forest) on TRN2.

Data-parallel over points across 8 NeuronCores. Per core, a hardware loop
(Tile For_i) processes batches of 1024 points:
  - DVE computes the spatial-hash table rows for all 8 trilinear corners of
    each point at each of the 16 levels (fp32-exact arithmetic for the
    mod-2^17 multiplies, int32 xor/and for the hash combine),
  - the per-corner feature pairs are fetched from a replicated bf16 copy of
    the full [8 trees x 16 levels x 131072 x 2] table in device DRAM via
    SWDGE indirect DMA (128 offsets per instruction, one per partition),
  - DVE applies the trilinear corner weights and accumulates the [N, 32]
    output, streamed back per batch.

The hash h = (x ^ y*P1 ^ z*P2) & (2^17-1) is computed without 32-bit integer
multiplies: only P mod 2^17 matters, and y*(P mod 2^17) is split into
(y%32)*(P mod 2^17) + (y//32)*((32*P) mod 2^17), both exact in fp32.

float->int casts on the vector engine round to nearest, so floor(pos) is
computed as cast(pos - 0.5), with the fractional weight w = pos - float(ip);
an off-by-one at exact-integer pos yields w=1.0 and the identical
interpolation result.
"""

import numpy as np

L = 16
F = 2
T = 1 << 17
N_TREES = 8
N = 2_000_000
RES = np.array([16, 22, 30, 41, 55, 75, 102, 139, 188, 256, 348, 472,
                642, 872, 1184, 1608], dtype=np.int64)
P1 = 2654435761
P2 = 805459861
MASK = T - 1
K1 = P1 & MASK            # y multiplier mod 2^17
K1H = (32 * P1) & MASK
K2 = P2 & MASK            # z multiplier mod 2^17
K2H = (32 * P2) & MASK

NCORES = 8
B = 1024                  # points per batch
NCOL = B // 128           # 8 point-columns per partition
NBATCH = 245              # batches per core
NPC = NBATCH * B          # 250880 points per core (padded)

_CACHE = {}


def _build_nc():
    import concourse.bass as bass
    import concourse.bacc as bacc
    import concourse.mybir as mybir
    import concourse.tile as tile
    from concourse.bass import IndirectOffsetOnAxis

    fp32 = mybir.dt.float32
    int32 = mybir.dt.int32
    bf16 = mybir.dt.bfloat16
    AO = mybir.AluOpType

    nc = bacc.Bacc("TRN2")
    # xb[b*128+p, d*NCOL+j] = x of point (b, j*128+p), dim d
    xb = nc.dram_tensor("xb", [NBATCH * 128, 3 * NCOL], fp32, kind="ExternalInput")
    # ftab[l*T + h] = feature pair for this core's tree (bf16)
    ftab = nc.dram_tensor("ftab", [L * T, F], bf16, kind="ExternalInput")
    out = nc.dram_tensor("out", [NPC, 2 * L], bf16, kind="ExternalOutput")

    with tile.TileContext(nc) as tc:
        with tc.tile_pool(name="sbuf", bufs=1) as pool:
            x_t = pool.tile([128, 3 * NCOL], fp32, tag="x")
            pos = pool.tile([128, 3 * NCOL], fp32, tag="pos")
            ipi = pool.tile([128, 3 * NCOL], int32, tag="ipi")
            ipf = pool.tile([128, 3 * NCOL], fp32, tag="ipf")
            w3 = pool.tile([128, 3 * NCOL], fp32, tag="w3")
            w3m = pool.tile([128, 3 * NCOL], fp32, tag="w3m")     # 1 - w
            hh = pool.tile([128, NCOL], fp32, tag="hh")           # scratch f
            hl = pool.tile([128, NCOL], fp32, tag="hl")
            tyf = pool.tile([128, 2 * NCOL], fp32, tag="tyf")     # y,z terms f
            tyi = pool.tile([128, 2 * NCOL], int32, tag="tyi")    # y0,z0 int
            tyi1 = pool.tile([128, 2 * NCOL], int32, tag="tyi1")  # y1,z1 int
            ix1 = pool.tile([128, NCOL], int32, tag="ix1")
            xy = pool.tile([128, 4 * NCOL], int32, tag="xy")      # x^y for 4 combos
            hc = pool.tile([128, NCOL], int32, tag="hc")          # corner hash scratch
            idx_t = pool.tile([128, 8 * NCOL], int32, tag="idx")
            g = pool.tile([128, 16 * NCOL], bf16, tag="g")
            gf = pool.tile([128, 16 * NCOL], fp32, tag="gf")
            wyz = pool.tile([128, 4 * NCOL], fp32, tag="wyz")
            w8 = pool.tile([128, 8 * NCOL], fp32, tag="w8")
            w16 = pool.tile([128, 16 * NCOL], fp32, tag="w16")
            tsum = pool.tile([128, 2 * NCOL], fp32, tag="tsum")
            acc = pool.tile([128, NCOL, 2 * L], fp32, tag="acc")
            accb = pool.tile([128, NCOL, 2 * L], bf16, tag="accb")

            with tc.For_i(0, NBATCH) as bi:
                nc.sync.dma_start(out=x_t[:], in_=xb[bass.ts(bi, 128), :])
                for l in range(L):
                    R = int(RES[l])
                    s = (R - 1) * 0.5
                    # pos = x*s + s ; ip = round(pos-0.5) ; w = pos - ip
                    nc.vector.tensor_scalar(out=pos[:], in0=x_t[:], scalar1=s,
                                            scalar2=s, op0=AO.mult, op1=AO.add)
                    nc.vector.tensor_scalar(out=ipf[:], in0=pos[:], scalar1=1.0,
                                            scalar2=-0.5, op0=AO.mult, op1=AO.add)
                    nc.vector.tensor_copy(out=ipi[:], in_=ipf[:])
                    nc.vector.tensor_copy(out=ipf[:], in_=ipi[:])
                    nc.vector.tensor_tensor(out=w3[:], in0=pos[:], in1=ipf[:],
                                            op=AO.subtract)
                    nc.vector.tensor_scalar(out=w3m[:], in0=w3[:], scalar1=-1.0,
                                            scalar2=1.0, op0=AO.mult, op1=AO.add)
                    # y/z hash terms: t = (i%32)*K + (i//32)*KH  (exact fp32)
                    for d, (KA, KB) in ((1, (K1, K1H)), (2, (K2, K2H))):
                        src = ipf[:, d * NCOL:(d + 1) * NCOL]
                        nc.vector.tensor_scalar(out=hh[:], in0=src, scalar1=0.03125,
                                                scalar2=-0.5, op0=AO.mult, op1=AO.add)
                        nc.vector.tensor_copy(out=hc[:], in_=hh[:])      # int floor
                        nc.vector.tensor_copy(out=hh[:], in_=hc[:])      # back to f
                        nc.vector.tensor_scalar(out=hl[:], in0=hh[:], scalar1=-32.0,
                                                scalar2=0.0, op0=AO.mult, op1=AO.add)
                        nc.vector.tensor_tensor(out=hl[:], in0=src, in1=hl[:],
                                                op=AO.add)               # i%32
                        nc.vector.tensor_scalar(out=hl[:], in0=hl[:], scalar1=float(KA),
                                                scalar2=0.0, op0=AO.mult, op1=AO.add)
                        nc.vector.tensor_scalar(out=hh[:], in0=hh[:], scalar1=float(KB),
                                                scalar2=0.0, op0=AO.mult, op1=AO.add)
                        dst = tyf[:, (d - 1) * NCOL:d * NCOL]
                        nc.vector.tensor_tensor(out=dst, in0=hl[:], in1=hh[:], op=AO.add)
                    nc.vector.tensor_copy(out=tyi[:], in_=tyf[:])
                    # +K for the +1 corners
                    nc.vector.tensor_scalar(out=tyi1[:, :NCOL], in0=tyi[:, :NCOL],
                                            scalar1=K1, scalar2=0, op0=AO.add, op1=AO.add)
                    nc.vector.tensor_scalar(out=tyi1[:, NCOL:], in0=tyi[:, NCOL:],
                                            scalar1=K2, scalar2=0, op0=AO.add, op1=AO.add)
                    nc.vector.tensor_scalar(out=ix1[:], in0=ipi[:, :NCOL], scalar1=1,
                                            scalar2=0, op0=AO.add, op1=AO.add)
                    # xy[dx*2+dy] = ix_dx ^ ty_dy
                    for dx, xsrc in ((0, ipi[:, :NCOL]), (1, ix1[:])):
                        for dy, ysrc in ((0, tyi[:, :NCOL]), (1, tyi1[:, :NCOL])):
                            nc.vector.tensor_tensor(
                                out=xy[:, (dx * 2 + dy) * NCOL:(dx * 2 + dy + 1) * NCOL],
                                in0=xsrc, in1=ysrc, op=AO.bitwise_xor)
                    # corners c = dx*4 + dy*2 + dz (matches OFFS ordering)
                    for dx in range(2):
                        for dy in range(2):
                            for dz in range(2):
                                c = dx * 4 + dy * 2 + dz
                                zsrc = tyi[:, NCOL:] if dz == 0 else tyi1[:, NCOL:]
                                nc.vector.tensor_tensor(
                                    out=hc[:],
                                    in0=xy[:, (dx * 2 + dy) * NCOL:(dx * 2 + dy + 1) * NCOL],
                                    in1=zsrc, op=AO.bitwise_xor)
                                nc.vector.tensor_scalar(out=idx_t[:, c * NCOL:(c + 1) * NCOL],
                                                        in0=hc[:],
                                                        scalar1=MASK, scalar2=l * T,
                                                        op0=AO.bitwise_and, op1=AO.bitwise_or)
                    # gathers: one 128-offset indirect DMA per (corner, column)
                    for q in range(8 * NCOL):
                        nc.gpsimd.indirect_dma_start(
                            out=g[:, 2 * q:2 * q + 2],
                            out_offset=None,
                            in_=ftab[:],
                            in_offset=IndirectOffsetOnAxis(ap=idx_t[:, q:q + 1], axis=0),
                        )
                    # weights: w8[c] = wx_dx * wy_dy * wz_dz
                    for dy in range(2):
                        ws = w3m if dy == 0 else w3
                        for dz in range(2):
                            zs = w3m if dz == 0 else w3
                            nc.vector.tensor_tensor(
                                out=wyz[:, (dy * 2 + dz) * NCOL:(dy * 2 + dz + 1) * NCOL],
                                in0=ws[:, NCOL:2 * NCOL], in1=zs[:, 2 * NCOL:3 * NCOL],
                                op=AO.mult)
                    for dx in range(2):
                        xs = w3m if dx == 0 else w3
                        for k in range(4):
                            c = dx * 4 + k
                            nc.vector.tensor_tensor(
                                out=w8[:, c * NCOL:(c + 1) * NCOL],
                                in0=xs[:, :NCOL], in1=wyz[:, k * NCOL:(k + 1) * NCOL],
                                op=AO.mult)
                    # duplicate each weight across the 2 features
                    w16v = w16[:].rearrange("p (q two) -> p q two", two=2)
                    w8v = w8[:].rearrange("p (q one) -> p q one", one=1)
                    nc.vector.tensor_copy(out=w16v[:, :, 0:1], in_=w8v)
                    nc.vector.tensor_copy(out=w16v[:, :, 1:2], in_=w8v)
                    # weighted sum over corners
                    nc.vector.tensor_copy(out=gf[:], in_=g[:])
                    nc.vector.tensor_tensor(out=gf[:], in0=gf[:], in1=w16[:], op=AO.mult)
                    nc.vector.tensor_tensor(out=tsum[:], in0=gf[:, :2 * NCOL],
                                            in1=gf[:, 2 * NCOL:4 * NCOL], op=AO.add)
                    for c in range(2, 8):
                        nc.vector.tensor_tensor(
                            out=tsum[:], in0=tsum[:],
                            in1=gf[:, c * 2 * NCOL:(c + 1) * 2 * NCOL], op=AO.add)
                    # tsum[p, j*2+f] -> acc[p, j, 2l+f]
                    nc.vector.tensor_copy(
                        out=acc[:, :, 2 * l:2 * l + 2],
                        in_=tsum[:].rearrange("p (j f) -> p j f", f=2))
                # out rows j*128+p <- acc[p, j, :]
                nc.vector.tensor_copy(out=accb[:], in_=acc[:])
                ov = out[bass.ts(bi, B), :].rearrange("(j p) f -> p j f", p=128)
                nc.sync.dma_start(out=ov, in_=accb[:])
    nc.compile()
    return nc


def _prep(block_x, params, block_inds):
    import ml_dtypes
    x = np.asarray(block_x, dtype=np.float32)
    inds = np.asarray(block_inds).astype(np.int64)
    n = x.shape[0]
    order = np.argsort(inds, kind="stable")
    counts = np.bincount(inds, minlength=N_TREES)
    starts = np.concatenate([[0], np.cumsum(counts)])
    bucket_idx = []   # global point ids handled by core c, in device row order
    xr = np.zeros((NCORES, NBATCH * 128, 3 * NCOL), dtype=np.float32)
    overflow = []     # (global ids) handled on host (bucket overflow; ~never)
    for c in range(NCORES):
        ids = order[starts[c]:starts[c + 1]]
        if len(ids) > NPC:
            overflow.append(ids[NPC:])
            ids = ids[:NPC]
        bucket_idx.append(ids)
        xc = np.zeros((NPC, 3), dtype=np.float32)
        xc[:len(ids)] = x[ids]
        xr[c] = np.ascontiguousarray(
            xc.reshape(NBATCH, NCOL, 128, 3).transpose(0, 2, 3, 1)
        ).reshape(NBATCH * 128, 3 * NCOL)
    ftabs = np.asarray(params, dtype=np.float32).reshape(
        N_TREES, L * T, F).astype(ml_dtypes.bfloat16)
    return xr, ftabs, bucket_idx, overflow, n


def _host_ref(block_x, params, ids, inds):
    """Exact numpy fallback for overflow points (rare)."""
    OFFS = np.stack(np.meshgrid([0, 1], [0, 1], [0, 1], indexing="ij"),
                    axis=-1).reshape(8, 3).astype(np.int32)
    x01 = np.asarray(block_x, np.float32)[ids] * np.float32(0.5) + np.float32(0.5)
    t = np.asarray(inds)[ids].astype(np.int64)
    out = np.empty((len(ids), 2 * L), dtype=np.float32)
    offs_b = OFFS.astype(bool)
    for l in range(L):
        R = int(RES[l])
        pos = x01 * np.float32(R - 1)
        p0 = np.floor(pos)
        w = pos - p0
        p0i = p0.astype(np.int32)
        corners = np.clip(p0i[:, None, :] + OFFS[None], 0, R - 1)
        cu = corners.astype(np.uint32)
        h = (cu[..., 0] ^ (cu[..., 1] * np.uint32(P1 & 0xFFFFFFFF))
             ^ (cu[..., 2] * np.uint32(P2 & 0xFFFFFFFF)))
        idx = (h & np.uint32(MASK)).astype(np.int64)
        feats = np.asarray(params, np.float32)[t[:, None], l, idx, :]
        wc = np.prod(np.where(offs_b[None], w[:, None, :],
                              np.float32(1.0) - w[:, None, :]), axis=-1)
        out[:, 2 * l:2 * l + 2] = np.einsum("nc,ncf->nf",
                                            wc.astype(np.float32), feats)
    return out


def kernel(block_x, params, block_inds):
    import time as _t
    from concourse.bass_utils import run_bass_kernel_spmd

    _t0 = _t.time()
    xr, ftabs, bucket_idx, overflow, n = _prep(block_x, params, block_inds)
    _t1 = _t.time()
    if "nc" not in _CACHE:
        _CACHE["nc"] = _build_nc()
    nc = _CACHE["nc"]
    in_maps = [{"xb": xr[c], "ftab": np.ascontiguousarray(ftabs[c])}
               for c in range(NCORES)]
    _t2 = _t.time()
    res = run_bass_kernel_spmd(nc, in_maps, core_ids=list(range(NCORES)))
    _t3 = _t.time()
    ou16 = np.empty((n, 2 * L), dtype=np.uint16)
    for c in range(NCORES):
        ids = bucket_idx[c]
        ou16[ids] = res.results[c]["out"][:len(ids)].view(np.uint16)
    full = (ou16.astype(np.uint32) << np.uint32(16)).view(np.float32)
    if overflow:
        ids = np.concatenate(overflow)
        full[ids] = _host_ref(block_x, params, ids, block_inds)
    import os
    if os.environ.get("KERNEL_VERBOSE"):
        print(f"[kernel] prep={_t1-_t0:.2f}s build={_t2-_t1:.2f}s "
              f"spmd={_t3-_t2:.2f}s post={_t.time()-_t3:.2f}s")
    return full


# revision 13
# speedup vs baseline: 6.5862x; 1.0994x over previous
"""LoTD forest encoding (NGP-style multi-level hash grid, 8-tree forest) on TRN2.

Data-parallel over points across 8 NeuronCores. Per core, a hardware loop
(Tile For_i) processes batches of 1024 points:
  - DVE computes the spatial-hash table rows for all 8 trilinear corners of
    each point at each of the 16 levels (fp32-exact arithmetic for the
    mod-2^17 multiplies, int32 xor/and for the hash combine),
  - the per-corner feature pairs are fetched from a replicated bf16 copy of
    the full [8 trees x 16 levels x 131072 x 2] table in device DRAM via
    SWDGE indirect DMA (128 offsets per instruction, one per partition),
  - DVE applies the trilinear corner weights and accumulates the [N, 32]
    output, streamed back per batch.

The hash h = (x ^ y*P1 ^ z*P2) & (2^17-1) is computed without 32-bit integer
multiplies: only P mod 2^17 matters, and y*(P mod 2^17) is split into
(y%32)*(P mod 2^17) + (y//32)*((32*P) mod 2^17), both exact in fp32.

float->int casts on the vector engine round to nearest, so floor(pos) is
computed as cast(pos - 0.5), with the fractional weight w = pos - float(ip);
an off-by-one at exact-integer pos yields w=1.0 and the identical
interpolation result.
"""

import numpy as np

L = 16
F = 2
T = 1 << 17
N_TREES = 8
N = 2_000_000
RES = np.array([16, 22, 30, 41, 55, 75, 102, 139, 188, 256, 348, 472,
                642, 872, 1184, 1608], dtype=np.int64)
P1 = 2654435761
P2 = 805459861
MASK = T - 1
K1 = P1 & MASK            # y multiplier mod 2^17
K1H = (32 * P1) & MASK
K2 = P2 & MASK            # z multiplier mod 2^17
K2H = (32 * P2) & MASK

NCORES = 8
B = 1024                  # points per batch
NCOL = B // 128           # 8 point-columns per partition
NBATCH = 245              # batches per core
NPC = NBATCH * B          # 250880 points per core (padded)

_CACHE = {}


def _build_nc():
    import concourse.bass as bass
    import concourse.bacc as bacc
    import concourse.mybir as mybir
    import concourse.tile as tile
    from concourse.bass import IndirectOffsetOnAxis

    fp32 = mybir.dt.float32
    int32 = mybir.dt.int32
    bf16 = mybir.dt.bfloat16
    AO = mybir.AluOpType

    nc = bacc.Bacc("TRN2")
    # xb[b*128+p, d*NCOL+j] = x of point (b, j*128+p), dim d
    xb = nc.dram_tensor("xb", [NBATCH * 128, 3 * NCOL], fp32, kind="ExternalInput")
    # ftab[l*T + h] = feature pair for this core's tree (bf16)
    ftab = nc.dram_tensor("ftab", [L * T, F], bf16, kind="ExternalInput")
    out = nc.dram_tensor("out", [NPC, 2 * L], bf16, kind="ExternalOutput")

    with tile.TileContext(nc) as tc:
        with tc.tile_pool(name="sbuf", bufs=1) as pool:
            x_t = pool.tile([128, 3 * NCOL], fp32, tag="x")
            pos = pool.tile([128, 3 * NCOL], fp32, tag="pos")
            ipi = pool.tile([128, 3 * NCOL], int32, tag="ipi")
            ipf = pool.tile([128, 3 * NCOL], fp32, tag="ipf")
            w3 = pool.tile([128, 3 * NCOL], fp32, tag="w3")
            w3m = pool.tile([128, 3 * NCOL], fp32, tag="w3m")     # 1 - w
            hh = pool.tile([128, NCOL], fp32, tag="hh")           # scratch f
            hl = pool.tile([128, NCOL], fp32, tag="hl")
            tyf = pool.tile([128, 2 * NCOL], fp32, tag="tyf")     # y,z terms f
            tyi = pool.tile([128, 2 * NCOL], int32, tag="tyi")    # y0,z0 int
            tyi1 = pool.tile([128, 2 * NCOL], int32, tag="tyi1")  # y1,z1 int
            ix1 = pool.tile([128, NCOL], int32, tag="ix1")
            xy = pool.tile([128, 4 * NCOL], int32, tag="xy")      # x^y for 4 combos
            hc = pool.tile([128, NCOL], int32, tag="hc")          # corner hash scratch
            idx_t = pool.tile([128, 8 * NCOL], int32, tag="idx")
            g = pool.tile([128, 16 * NCOL], bf16, tag="g")
            gf = pool.tile([128, 16 * NCOL], fp32, tag="gf")
            wyz = pool.tile([128, 4 * NCOL], fp32, tag="wyz")
            w8 = pool.tile([128, 8 * NCOL], fp32, tag="w8")
            w16 = pool.tile([128, 16 * NCOL], fp32, tag="w16")
            tsum = pool.tile([128, 2 * NCOL], fp32, tag="tsum")
            acc = pool.tile([128, NCOL, 2 * L], fp32, tag="acc")
            accb = pool.tile([128, NCOL, 2 * L], bf16, tag="accb")

            with tc.For_i(0, NBATCH) as bi:
                nc.sync.dma_start(out=x_t[:], in_=xb[bass.ts(bi, 128), :])
                for l in range(L):
                    R = int(RES[l])
                    s = (R - 1) * 0.5
                    # pos = x*s + s ; ip = round(pos-0.5) ; w = pos - ip
                    nc.vector.tensor_scalar(out=pos[:], in0=x_t[:], scalar1=s,
                                            scalar2=s, op0=AO.mult, op1=AO.add)
                    nc.vector.tensor_scalar(out=ipf[:], in0=pos[:], scalar1=1.0,
                                            scalar2=-0.5, op0=AO.mult, op1=AO.add)
                    nc.vector.tensor_copy(out=ipi[:], in_=ipf[:])
                    nc.vector.tensor_copy(out=ipf[:], in_=ipi[:])
                    nc.vector.tensor_tensor(out=w3[:], in0=pos[:], in1=ipf[:],
                                            op=AO.subtract)
                    nc.vector.tensor_scalar(out=w3m[:], in0=w3[:], scalar1=-1.0,
                                            scalar2=1.0, op0=AO.mult, op1=AO.add)
                    # y/z hash terms: t = (i%32)*K + (i//32)*KH  (exact fp32)
                    for d, (KA, KB) in ((1, (K1, K1H)), (2, (K2, K2H))):
                        src = ipf[:, d * NCOL:(d + 1) * NCOL]
                        nc.vector.tensor_scalar(out=hh[:], in0=src, scalar1=0.03125,
                                                scalar2=-0.5, op0=AO.mult, op1=AO.add)
                        nc.vector.tensor_copy(out=hc[:], in_=hh[:])      # int floor
                        nc.vector.tensor_copy(out=hh[:], in_=hc[:])      # back to f
                        nc.vector.tensor_scalar(out=hl[:], in0=hh[:], scalar1=-32.0,
                                                scalar2=0.0, op0=AO.mult, op1=AO.add)
                        nc.vector.tensor_tensor(out=hl[:], in0=src, in1=hl[:],
                                                op=AO.add)               # i%32
                        nc.vector.tensor_scalar(out=hl[:], in0=hl[:], scalar1=float(KA),
                                                scalar2=0.0, op0=AO.mult, op1=AO.add)
                        nc.vector.tensor_scalar(out=hh[:], in0=hh[:], scalar1=float(KB),
                                                scalar2=0.0, op0=AO.mult, op1=AO.add)
                        dst = tyf[:, (d - 1) * NCOL:d * NCOL]
                        nc.vector.tensor_tensor(out=dst, in0=hl[:], in1=hh[:], op=AO.add)
                    nc.vector.tensor_copy(out=tyi[:], in_=tyf[:])
                    # +K for the +1 corners
                    nc.vector.tensor_scalar(out=tyi1[:, :NCOL], in0=tyi[:, :NCOL],
                                            scalar1=K1, scalar2=0, op0=AO.add, op1=AO.add)
                    nc.vector.tensor_scalar(out=tyi1[:, NCOL:], in0=tyi[:, NCOL:],
                                            scalar1=K2, scalar2=0, op0=AO.add, op1=AO.add)
                    nc.vector.tensor_scalar(out=ix1[:], in0=ipi[:, :NCOL], scalar1=1,
                                            scalar2=0, op0=AO.add, op1=AO.add)
                    # xy[dx*2+dy] = ix_dx ^ ty_dy
                    for dx, xsrc in ((0, ipi[:, :NCOL]), (1, ix1[:])):
                        for dy, ysrc in ((0, tyi[:, :NCOL]), (1, tyi1[:, :NCOL])):
                            nc.vector.tensor_tensor(
                                out=xy[:, (dx * 2 + dy) * NCOL:(dx * 2 + dy + 1) * NCOL],
                                in0=xsrc, in1=ysrc, op=AO.bitwise_xor)
                    # corners c = dx*4 + dy*2 + dz (matches OFFS ordering)
                    for dx in range(2):
                        for dy in range(2):
                            for dz in range(2):
                                c = dx * 4 + dy * 2 + dz
                                zsrc = tyi[:, NCOL:] if dz == 0 else tyi1[:, NCOL:]
                                nc.vector.tensor_tensor(
                                    out=hc[:],
                                    in0=xy[:, (dx * 2 + dy) * NCOL:(dx * 2 + dy + 1) * NCOL],
                                    in1=zsrc, op=AO.bitwise_xor)
                                nc.vector.tensor_scalar(out=idx_t[:, c * NCOL:(c + 1) * NCOL],
                                                        in0=hc[:],
                                                        scalar1=MASK, scalar2=l * T,
                                                        op0=AO.bitwise_and, op1=AO.bitwise_or)
                    # gathers: one 128-offset indirect DMA per (corner, column)
                    for q in range(8 * NCOL):
                        nc.gpsimd.indirect_dma_start(
                            out=g[:, 2 * q:2 * q + 2],
                            out_offset=None,
                            in_=ftab[:],
                            in_offset=IndirectOffsetOnAxis(ap=idx_t[:, q:q + 1], axis=0),
                        )
                    # weights: w8[c] = wx_dx * wy_dy * wz_dz
                    for dy in range(2):
                        ws = w3m if dy == 0 else w3
                        for dz in range(2):
                            zs = w3m if dz == 0 else w3
                            nc.vector.tensor_tensor(
                                out=wyz[:, (dy * 2 + dz) * NCOL:(dy * 2 + dz + 1) * NCOL],
                                in0=ws[:, NCOL:2 * NCOL], in1=zs[:, 2 * NCOL:3 * NCOL],
                                op=AO.mult)
                    for dx in range(2):
                        xs = w3m if dx == 0 else w3
                        for k in range(4):
                            c = dx * 4 + k
                            nc.vector.tensor_tensor(
                                out=w8[:, c * NCOL:(c + 1) * NCOL],
                                in0=xs[:, :NCOL], in1=wyz[:, k * NCOL:(k + 1) * NCOL],
                                op=AO.mult)
                    # duplicate each weight across the 2 features
                    w16v = w16[:].rearrange("p (q two) -> p q two", two=2)
                    w8v = w8[:].rearrange("p (q one) -> p q one", one=1)
                    nc.vector.tensor_copy(out=w16v[:, :, 0:1], in_=w8v)
                    nc.vector.tensor_copy(out=w16v[:, :, 1:2], in_=w8v)
                    # weighted sum over corners
                    nc.vector.tensor_copy(out=gf[:], in_=g[:])
                    nc.vector.tensor_tensor(out=gf[:], in0=gf[:], in1=w16[:], op=AO.mult)
                    nc.vector.tensor_tensor(out=tsum[:], in0=gf[:, :2 * NCOL],
                                            in1=gf[:, 2 * NCOL:4 * NCOL], op=AO.add)
                    for c in range(2, 8):
                        nc.vector.tensor_tensor(
                            out=tsum[:], in0=tsum[:],
                            in1=gf[:, c * 2 * NCOL:(c + 1) * 2 * NCOL], op=AO.add)
                    # tsum[p, j*2+f] -> acc[p, j, 2l+f]
                    nc.vector.tensor_copy(
                        out=acc[:, :, 2 * l:2 * l + 2],
                        in_=tsum[:].rearrange("p (j f) -> p j f", f=2))
                # out rows j*128+p <- acc[p, j, :]
                nc.vector.tensor_copy(out=accb[:], in_=acc[:])
                ov = out[bass.ts(bi, B), :].rearrange("(j p) f -> p j f", p=128)
                nc.sync.dma_start(out=ov, in_=accb[:])
    nc.compile()
    return nc


def _prep(block_x, params, block_inds):
    import ml_dtypes
    x = np.asarray(block_x, dtype=np.float32)
    inds = np.asarray(block_inds).astype(np.int64)
    n = x.shape[0]
    order = np.argsort(inds, kind="stable")
    counts = np.bincount(inds, minlength=N_TREES)
    starts = np.concatenate([[0], np.cumsum(counts)])
    bucket_idx = []   # global point ids handled by core c, in device row order
    xr = np.zeros((NCORES, NBATCH * 128, 3 * NCOL), dtype=np.float32)
    overflow = []     # (global ids) handled on host (bucket overflow; ~never)
    for c in range(NCORES):
        ids = order[starts[c]:starts[c + 1]]
        if len(ids) > NPC:
            overflow.append(ids[NPC:])
            ids = ids[:NPC]
        bucket_idx.append(ids)
        xc = np.zeros((NPC, 3), dtype=np.float32)
        xc[:len(ids)] = x[ids]
        xr[c] = np.ascontiguousarray(
            xc.reshape(NBATCH, NCOL, 128, 3).transpose(0, 2, 3, 1)
        ).reshape(NBATCH * 128, 3 * NCOL)
    ftabs = np.asarray(params, dtype=np.float32).reshape(
        N_TREES, L * T, F).astype(ml_dtypes.bfloat16)
    return xr, ftabs, bucket_idx, overflow, n


def _host_ref(block_x, params, ids, inds):
    """Exact numpy fallback for overflow points (rare)."""
    OFFS = np.stack(np.meshgrid([0, 1], [0, 1], [0, 1], indexing="ij"),
                    axis=-1).reshape(8, 3).astype(np.int32)
    x01 = np.asarray(block_x, np.float32)[ids] * np.float32(0.5) + np.float32(0.5)
    t = np.asarray(inds)[ids].astype(np.int64)
    out = np.empty((len(ids), 2 * L), dtype=np.float32)
    offs_b = OFFS.astype(bool)
    for l in range(L):
        R = int(RES[l])
        pos = x01 * np.float32(R - 1)
        p0 = np.floor(pos)
        w = pos - p0
        p0i = p0.astype(np.int32)
        corners = np.clip(p0i[:, None, :] + OFFS[None], 0, R - 1)
        cu = corners.astype(np.uint32)
        h = (cu[..., 0] ^ (cu[..., 1] * np.uint32(P1 & 0xFFFFFFFF))
             ^ (cu[..., 2] * np.uint32(P2 & 0xFFFFFFFF)))
        idx = (h & np.uint32(MASK)).astype(np.int64)
        feats = np.asarray(params, np.float32)[t[:, None], l, idx, :]
        wc = np.prod(np.where(offs_b[None], w[:, None, :],
                              np.float32(1.0) - w[:, None, :]), axis=-1)
        out[:, 2 * l:2 * l + 2] = np.einsum("nc,ncf->nf",
                                            wc.astype(np.float32), feats)
    return out


def kernel(block_x, params, block_inds):
    import time as _t
    from concourse.bass_utils import run_bass_kernel_spmd

    _t0 = _t.time()
    xr, ftabs, bucket_idx, overflow, n = _prep(block_x, params, block_inds)
    _t1 = _t.time()
    if "nc" not in _CACHE:
        _CACHE["nc"] = _build_nc()
    nc = _CACHE["nc"]
    in_maps = [{"xb": xr[c], "ftab": np.ascontiguousarray(ftabs[c])}
               for c in range(NCORES)]
    _t2 = _t.time()
    res = run_bass_kernel_spmd(nc, in_maps, core_ids=list(range(NCORES)))
    _t3 = _t.time()
    full = np.zeros((n, 2 * L), dtype=np.float32)
    hi16 = full.view(np.uint16).reshape(n, 2 * L, 2)[:, :, 1]  # little-endian high half
    for c in range(NCORES):
        ids = bucket_idx[c]
        hi16[ids] = res.results[c]["out"][:len(ids)].view(np.uint16)
    if overflow:
        ids = np.concatenate(overflow)
        full[ids] = _host_ref(block_x, params, ids, block_inds)
    import os
    if os.environ.get("KERNEL_VERBOSE"):
        print(f"[kernel] prep={_t1-_t0:.2f}s build={_t2-_t1:.2f}s "
              f"spmd={_t3-_t2:.2f}s post={_t.time()-_t3:.2f}s")
    return full
